# revision 1
# baseline (speedup 1.0000x reference)
"""GCN layer (x @ W -> edge gather/scale/scatter-add -> +bias, relu) on 8 NeuronCores.

Measured: ~650 us/iteration on 8 axon-tunneled trn2 cores, rel err 4.5e-07
vs the fp32 jax reference (timing via on-device repeat slope; the axon RPC
adds ~95 ms/call that the slope cancels).

Strategy (per sharding hint):
  - Shard nodes across 8 cores (6250 rows each). Each core computes its local
    xw = x_shard @ W on the PE (host pre-transposes x so K lands on
    partitions), then an AllGather builds the full xw table [50000, 64] in
    every core's DRAM.
  - Edges are partitioned by destination shard on the host, sorted by
    destination window (128 nodes), grouped into 128-edge groups that each
    target a single window.  Per group the core:
      * dma_gather's the 128 source rows (256B each) from the xw table
        (SWDGE custom gather, 4 SWDGE queues round-robin -- queue depth is
        the gather throughput lever: 33 GB/s at q=1 vs 323 GB/s at q=4)
      * builds a one-hot*val matrix [128e, 128d] with one fused DVE
        tensor_scalar (iota == dstloc) * val   (ohk=0 path; ohk>0 selects a
        bulk tensor_tensor builder, measured slower in situ)
      * accumulates psum[64f, 128d] += msgs[128e, 64f].T @ onehot on the PE
        (messages are the 64-col stationary operand, onehot streams)
    Window accumulation ends with a fused bias+relu on the scalar engine
    (bias is per-partition in the transposed layout) into an output staging
    tile; one DMA stores outT [64, 6250] and the host transposes.
  - int16 gather indices only address <32768 rows, so edges are split into a
    "low" stream (src < 25000) and "high" stream (src >= 25000), gathered
    from the matching half of the xw table.  Group counts per (window, half)
    are padded to the max over cores (~+13% edges) so all 8 cores run one
    SPMD program (run_bass_kernel_spmd shares a single instruction stream).
  - Host reassembles: out = concat(outT_c.T).
"""

import os
import sys

import numpy as np


def _ensure_concourse():
    try:
        import concourse  # noqa: F401
        return
    except ImportError:
        pass
    for p in ("/opt/trn_rl_repo", "/root/.axon_site/_ro/trn_rl_repo"):
        if os.path.isdir(p):
            sys.path.insert(0, p)
            try:
                import concourse  # noqa: F401
                return
            except ImportError:
                sys.path.pop(0)
    raise ImportError("concourse (bass) not found")


_ensure_concourse()

import concourse.bacc as bacc  # noqa: E402
import concourse.mybir as mybir  # noqa: E402
import concourse.tile as tile  # noqa: E402
from concourse import bass_utils  # noqa: E402

F32 = mybir.dt.float32
I16 = mybir.dt.int16
I32 = mybir.dt.int32


def _cdiv(a, b):
    return -(-a // b)


def preprocess(edge_src, edge_dst, edge_vals, *, n_nodes, cores, win):
    """Partition/sort/pad edges into per-core low/high streams.

    Returns a dict with SPMD-uniform structure (G arrays) and per-core data
    arrays laid out exactly as the device consumes them.
    """
    shard = n_nodes // cores
    nwin = _cdiv(shard, win)
    half = n_nodes // 2

    src = np.asarray(edge_src).astype(np.int64)
    dst = np.asarray(edge_dst).astype(np.int64)
    vals = np.asarray(edge_vals).astype(np.float32)
    e = src.shape[0]

    core = dst // shard
    dl = dst - core * shard
    w = dl // win
    h = (src >= half).astype(np.int64)
    key = (core * nwin + w) * 2 + h

    order = np.argsort(key, kind="stable")
    ks = key[order]
    src_s = src[order]
    dloc_s = (dl - w * win)[order].astype(np.float32)
    v_s = vals[order]
    c_s = core[order]
    w_s = w[order]
    h_s = h[order]

    nbuck = cores * nwin * 2
    sizes = np.bincount(key, minlength=nbuck)
    starts = np.concatenate(([0], np.cumsum(sizes)))[:-1]
    rank = np.arange(e, dtype=np.int64) - starts[ks]

    # groups per (window, half): max over cores
    cnt = sizes.reshape(cores, nwin, 2)
    G = _cdiv(cnt, 128).max(axis=0)  # [nwin, 2]
    glo, ghi = G[:, 0], G[:, 1]
    cum_lo = np.concatenate(([0], np.cumsum(glo)))  # group offsets per window
    cum_hi = np.concatenate(([0], np.cumsum(ghi)))
    gtot_lo, gtot_hi = int(cum_lo[-1]), int(cum_hi[-1])
    nlo, nhi = gtot_lo * 128, gtot_hi * 128

    idx_lo = np.zeros((cores, max(nlo, 1)), np.int16)
    dst_lo = np.zeros((cores, max(nlo, 1)), np.float32)
    val_lo = np.zeros((cores, max(nlo, 1)), np.float32)
    idx_hi = np.zeros((cores, max(nhi, 1)), np.int16)
    dst_hi = np.zeros((cores, max(nhi, 1)), np.float32)
    val_hi = np.zeros((cores, max(nhi, 1)), np.float32)

    m = h_s == 0
    pos = cum_lo[w_s[m]] * 128 + rank[m]
    idx_lo[c_s[m], pos] = src_s[m].astype(np.int16)
    dst_lo[c_s[m], pos] = dloc_s[m]
    val_lo[c_s[m], pos] = v_s[m]

    m = h_s == 1
    pos = cum_hi[w_s[m]] * 128 + rank[m]
    idx_hi[c_s[m], pos] = (src_s[m] - half).astype(np.int16)
    dst_hi[c_s[m], pos] = dloc_s[m]
    val_hi[c_s[m], pos] = v_s[m]

    def idx_layout(a, n):
        # logical position i -> [i % 16, i // 16], replicated to 128 partitions
        if n == 0:
            return None
        blk = a[:n].reshape(-1, 16).T  # [16, n/16]
        return np.ascontiguousarray(np.tile(blk, (8, 1)))  # [128, n/16]

    def grp_layout(a, n):
        # position g*128+p -> [p, g]
        if n == 0:
            return None
        return np.ascontiguousarray(a[:n].reshape(-1, 128).T)  # [128, G]

    return dict(
        shard=shard,
        nwin=nwin,
        half=half,
        glo=glo,
        ghi=ghi,
        cum_lo=cum_lo,
        cum_hi=cum_hi,
        gtot_lo=gtot_lo,
        gtot_hi=gtot_hi,
        nlo=nlo,
        nhi=nhi,
        idx_lo=[idx_layout(idx_lo[c], nlo) for c in range(cores)],
        dst_lo=[grp_layout(dst_lo[c], nlo) for c in range(cores)],
        val_lo=[grp_layout(val_lo[c], nlo) for c in range(cores)],
        idx_hi=[idx_layout(idx_hi[c], nhi) for c in range(cores)],
        dst_hi=[grp_layout(dst_hi[c], nhi) for c in range(cores)],
        val_hi=[grp_layout(val_hi[c], nhi) for c in range(cores)],
    )


def build_program(meta, *, n_nodes, din, dout, cores, win, maxb=64,
                  msgs_bufs=2, gp_build=0, sc_ps_bufs=4, debug_mode=None,
                  ohk=16, oh_bufs=3, act_build=0,
                  debug_skip_gemm=False, debug_skip_scatter=False, repeat=1):
    """Build the SPMD Bass program. Returns (nc, input_names)."""
    shard = meta["shard"]
    nwin = meta["nwin"]
    half = meta["half"]
    glo, ghi = meta["glo"], meta["ghi"]
    cum_lo, cum_hi = meta["cum_lo"], meta["cum_hi"]
    gtot_lo, gtot_hi = meta["gtot_lo"], meta["gtot_hi"]
    nlo, nhi = meta["nlo"], meta["nhi"]
    kch = _cdiv(din, 128)
    nr = _cdiv(shard, 128)

    nc = bacc.Bacc("TRN2", target_bir_lowering=False, debug=False,
                   num_devices=cores, num_swdge_queues=4)

    t_xT = nc.dram_tensor("xT", [din, shard], F32, kind="ExternalInput")
    t_w = nc.dram_tensor("w", [din, dout], F32, kind="ExternalInput")
    t_bias = nc.dram_tensor("bias", [dout, 1], F32, kind="ExternalInput")
    t_idx = {}
    t_dst = {}
    t_val = {}
    if nlo:
        t_idx["lo"] = nc.dram_tensor("idx_lo", [128, nlo // 16], I16, kind="ExternalInput")
        t_dst["lo"] = nc.dram_tensor("dst_lo", [128, gtot_lo], F32, kind="ExternalInput")
        t_val["lo"] = nc.dram_tensor("val_lo", [128, gtot_lo], F32, kind="ExternalInput")
    if nhi:
        t_idx["hi"] = nc.dram_tensor("idx_hi", [128, nhi // 16], I16, kind="ExternalInput")
        t_dst["hi"] = nc.dram_tensor("dst_hi", [128, gtot_hi], F32, kind="ExternalInput")
        t_val["hi"] = nc.dram_tensor("val_hi", [128, gtot_hi], F32, kind="ExternalInput")
    t_out = nc.dram_tensor("outT", [dout, shard], F32, kind="ExternalOutput")

    if debug_skip_gemm:
        t_xw_full = nc.dram_tensor("xw_full", [n_nodes, dout], F32,
                                   kind="ExternalInput")
    else:
        t_xw_shard = nc.dram_tensor("xw_shard", [shard, dout], F32)
        t_xw_full = nc.dram_tensor("xw_full", [n_nodes, dout], F32,
                                   addr_space="Shared" if cores > 4 else "Local")

    with tile.TileContext(nc) as tc:
        with (
            tc.tile_pool(name="const", bufs=1) as constp,
            tc.tile_pool(name="xt", bufs=1) as xtp,
            tc.tile_pool(name="stage", bufs=1) as stagep,
            tc.tile_pool(name="meta", bufs=1) as metap,
            tc.tile_pool(name="msgs_lo", bufs=msgs_bufs) as mlp,
            tc.tile_pool(name="msgs_hi", bufs=msgs_bufs) as mhp,
            tc.tile_pool(name="oh", bufs=oh_bufs) as ohp,
            tc.tile_pool(name="gemm_ps", bufs=2, space="PSUM") as gpsp,
            tc.tile_pool(name="sc_ps", bufs=sc_ps_bufs, space="PSUM") as spsp,
        ):
            # ---- constants ----
            # iota repeated max(ohk,1) times: iota_rep[p, k*win + j] = j
            iw = max(ohk, 1)
            iota_i = constp.tile([128, iw * win], I32)
            nc.gpsimd.iota(iota_i[:], pattern=[[0, iw], [1, win]], base=0,
                           channel_multiplier=0)
            iota_f = constp.tile([128, iw * win], F32)
            nc.vector.tensor_copy(iota_f[:], iota_i[:])
            bias_sb = constp.tile([dout, 1], F32)
            nc.sync.dma_start(bias_sb[:], t_bias[:])
            w_sb = constp.tile([128, kch * dout], F32)
            nc.sync.dma_start(
                w_sb[:].rearrange("p (k e) -> p k e", e=dout),
                t_w[:].rearrange("(k p) e -> p k e", p=128),
            )

            # ---- metadata loads ----
            sb_idx = {}
            sb_dst = {}
            sb_val = {}
            for s in t_idx:
                sb_idx[s] = metap.tile(list(t_idx[s].shape), I16, tag=f"idx_{s}", name=f"sb_idx_{s}")
                nc.sync.dma_start(sb_idx[s][:], t_idx[s][:])
                sb_dst[s] = metap.tile(list(t_dst[s].shape), F32, tag=f"dst_{s}", name=f"sb_dst_{s}")
                nc.sync.dma_start(sb_dst[s][:], t_dst[s][:])
                sb_val[s] = metap.tile(list(t_val[s].shape), F32, tag=f"val_{s}", name=f"sb_val_{s}")
                nc.sync.dma_start(sb_val[s][:], t_val[s][:])
            sb_negd = {}
            sb_negv = {}
            if act_build:
                for s in t_idx:
                    sb_negd[s] = metap.tile(list(t_dst[s].shape), F32,
                                            tag=f"negd_{s}", name=f"sb_negd_{s}")
                    nc.vector.tensor_scalar(
                        sb_negd[s][:], sb_dst[s][:], -1.0, None,
                        mybir.AluOpType.mult)
                    sb_negv[s] = metap.tile(list(t_val[s].shape), F32,
                                            tag=f"negv_{s}", name=f"sb_negv_{s}")
                    nc.vector.tensor_scalar(
                        sb_negv[s][:], sb_val[s][:], -1.0, None,
                        mybir.AluOpType.mult)

            # ---- per-iteration body (repeat>1 used only for timing) ----
            def emit_body(rep):
                # ---- local GEMM: xw_shard = x_shard @ W ----
                if debug_skip_gemm:
                    xt_sb = None
                else:
                    xt_sb = []
                    for k in range(kch):
                        kp = min(128, din - k * 128)
                        xt = xtp.tile([kp, shard], F32, tag=f"xt{k}")
                        nc.sync.dma_start(xt[:], t_xT[k * 128:k * 128 + kp, :])
                        xt_sb.append(xt)
                    xw_stage = stagep.tile([128, nr * dout], F32, tag="xw_stage")
                    for r in range(nr):
                        rw = min(128, shard - r * 128)
                        ps = gpsp.tile([rw, dout], F32, tag="gemm_ps")
                        for k in range(kch):
                            nc.tensor.matmul(
                                ps[:],
                                xt_sb[k][:, r * 128:r * 128 + rw],
                                w_sb[:xt_sb[k].shape[0], k * dout:(k + 1) * dout],
                                start=(k == 0),
                                stop=(k == kch - 1),
                            )
                        nc.scalar.activation(
                        xw_stage[:rw, r * dout:(r + 1) * dout], ps[:],
                        mybir.ActivationFunctionType.Copy)
                    # store xw_shard (row-major) then AllGather
                    nfull = shard // 128
                    nc.sync.dma_start(
                        t_xw_shard[: nfull * 128, :].rearrange("(r p) e -> p r e", p=128),
                        xw_stage[:, : nfull * dout].rearrange("p (r e) -> p r e", e=dout),
                    )
                    if shard > nfull * 128:
                        rw = shard - nfull * 128
                        nc.sync.dma_start(
                            t_xw_shard[nfull * 128:, :],
                            xw_stage[:rw, nfull * dout:(nfull + 1) * dout],
                        )
                    nc.gpsimd.collective_compute(
                        "AllGather",
                        mybir.AluOpType.bypass,
                        replica_groups=[list(range(cores))],
                        ins=[t_xw_shard[:]],
                        outs=[t_xw_full[:]],
                    )

                # ---- scatter phase ----
                in_ap = {}
                if nlo:
                    in_ap["lo"] = t_xw_full[0:half, :]
                if nhi:
                    in_ap["hi"] = t_xw_full[half:n_nodes, :]
                gtot = {"lo": gtot_lo, "hi": gtot_hi}
                nbatch = {s: _cdiv(gtot[s], maxb) for s in in_ap}
                pool = {"lo": mlp, "hi": mhp}
                msgs_buf = {s: [None] * nbatch[s] for s in in_ap}
                qctr = [0]
                noh = {s: _cdiv(gtot[s], max(ohk, 1)) for s in in_ap}
                oh_buf = {s: [None] * noh[s] for s in in_ap}
                scaled = {s: [False] * nbatch[s] for s in in_ap}

                def emit_oh(s, c):
                    g0 = c * ohk
                    gn = min(ohk, gtot[s] - g0)
                    buf = ohp.tile([128, gn * win], F32, tag="oh",
                                   name=f"oh_{s}_{c}_r{rep}")
                    nc.vector.tensor_tensor(
                        buf[:].rearrange("p (k j) -> p k j", j=win),
                        iota_f[:, :gn * win].rearrange("p (k j) -> p k j", j=win),
                        sb_dst[s][:, g0:g0 + gn].broadcast_to([128, gn, win]),
                        op=mybir.AluOpType.is_equal,
                    )
                    oh_buf[s][c] = buf

                def emit_gather(s, b):
                    g0 = b * maxb
                    gn = min(maxb, gtot[s] - g0)
                    n_idx = gn * 128
                    buf = pool[s].tile([128, gn * dout], F32, tag=f"msgs_{s}", name=f"msgs_{s}_{b}_r{rep}")
                    nc.gpsimd.dma_gather(
                        buf[:].rearrange("p (c e) -> p c e", e=dout),
                        in_ap[s],
                        sb_idx[s][:, g0 * 8:(g0 + gn) * 8],
                        n_idx,
                        n_idx,
                        dout,
                        single_packet=False,
                        queue_num=qctr[0] % 4,
                    )
                    qctr[0] += 1
                    msgs_buf[s][b] = buf

                def emit_scale(s, b):
                    # fold edge values into the messages: one bulk multiply.
                    # Deferred to first consumption so the DVE doesn't
                    # head-of-line block on the gather DMA.
                    g0 = b * maxb
                    gn = min(maxb, gtot[s] - g0)
                    buf = msgs_buf[s][b]
                    nc.vector.tensor_tensor(
                        buf[:].rearrange("p (c e) -> p c e", e=dout),
                        buf[:].rearrange("p (c e) -> p c e", e=dout),
                        sb_val[s][:, g0:g0 + gn].broadcast_to([128, gn, dout]),
                        op=mybir.AluOpType.mult,
                    )
                    scaled[s][b] = True

                out_stage = stagep.tile([dout, shard], F32, tag="out_stage")
                cum = {"lo": cum_lo, "hi": cum_hi}
                if debug_skip_scatter == "gather_only":
                    # gathers only; consume each batch with one cheap DVE add
                    acc = stagep.tile([128, dout], F32, tag="dbg_acc")
                    nc.vector.memset(acc[:], 0.0)
                    for s in in_ap:
                        for b in range(nbatch[s]):
                            emit_gather(s, b)
                            nc.vector.tensor_tensor(
                                acc[:], acc[:], msgs_buf[s][b][:, :dout],
                                op=mybir.AluOpType.add)
                    nc.vector.tensor_copy(out_stage[:, :dout], acc[:dout, :dout])
                    nc.sync.dma_start(t_out[:], out_stage[:])
                    return
                if debug_skip_scatter:
                    # debug: outT = xw_full[core-shard].T via strided DMA read
                    nc.sync.dma_start(
                        out_stage[:],
                        t_xw_full[0:shard, :].rearrange("n e -> e n"))
                    nc.sync.dma_start(t_out[:], out_stage[:])
                    nwin_eff = 0
                else:
                    nwin_eff = nwin
                for wi in range(nwin_eff):
                    ww = min(win, shard - wi * win)
                    spans = [(s, int(cum[s][wi]), int(cum[s][wi + 1])) for s in in_ap]
                    ngrp = sum(g1 - g0 for _, g0, g1 in spans)
                    if ngrp == 0:
                        # no edges anywhere for this window: bias + relu of zero
                        zps = spsp.tile([dout, win], F32, tag="sc_ps")
                        nc.vector.memset(zps[:], 0.0)
                        nc.scalar.activation(
                            out_stage[:, wi * win:wi * win + ww], zps[:, :ww],
                            mybir.ActivationFunctionType.Relu, bias=bias_sb[:],
                        )
                        continue
                    ps = spsp.tile([dout, win], F32, tag="sc_ps")
                    gi = 0
                    for s, g0, g1 in spans:
                        for g in range(g0, g1):
                            b, j = g // maxb, g % maxb
                            if msgs_buf[s][b] is None:
                                emit_gather(s, b)
                            if debug_mode == "const_oh":
                                oh_ap = iota_f[:, :win]
                            elif ohk == 0:
                                oh = ohp.tile([128, win], F32, tag="oh")
                                if act_build and gi % act_build == act_build - 1:
                                    # scalar-engine build (exact for int iota):
                                    # oh = val * relu(1 - |iota - dst|)
                                    ab = ohp.tile([128, win], F32, tag="abst")
                                    nc.scalar.activation(
                                        ab[:], iota_f[:, :win],
                                        mybir.ActivationFunctionType.Abs,
                                        bias=sb_negd[s][:, g:g + 1])
                                    nc.scalar.activation(
                                        oh[:], ab[:],
                                        mybir.ActivationFunctionType.Relu,
                                        bias=sb_val[s][:, g:g + 1],
                                        scale=sb_negv[s][:, g:g + 1])
                                else:
                                    # per-group fused build on DVE
                                    nc.vector.tensor_scalar(
                                        oh[:],
                                        iota_f[:, :win],
                                        sb_dst[s][:, g:g + 1],
                                        sb_val[s][:, g:g + 1],
                                        mybir.AluOpType.is_equal,
                                        mybir.AluOpType.mult,
                                    )
                                oh_ap = oh[:]
                            else:
                                if not scaled[s][b]:
                                    emit_scale(s, b)
                                oc, ojj = g // ohk, g % ohk
                                if oh_buf[s][oc] is None:
                                    emit_oh(s, oc)
                                oh_ap = oh_buf[s][oc][:, ojj * win:(ojj + 1) * win]
                            if debug_mode == "no_mm":
                                if gi == 0:
                                    nc.tensor.matmul(
                                        ps[:], msgs_buf[s][b][:, j * dout:(j + 1) * dout],
                                        oh_ap, start=True, stop=True)
                            else:
                                nc.tensor.matmul(
                                    ps[:],
                                    msgs_buf[s][b][:, j * dout:(j + 1) * dout],
                                    oh_ap,
                                    start=(gi == 0),
                                    stop=(gi == ngrp - 1),
                                )
                            gi += 1
                    nc.scalar.activation(
                        out_stage[:, wi * win:wi * win + ww], ps[:, :ww],
                        mybir.ActivationFunctionType.Relu, bias=bias_sb[:],
                    )
                nc.sync.dma_start(t_out[:], out_stage[:])

            for _rep in range(repeat):
                emit_body(_rep)

    nc.compile()
    return nc


def run(inputs, *, n_nodes, n_edges, din, dout, cores, win=128, maxb=64,
        msgs_bufs=2, gp_build=0, sc_ps_bufs=4, debug_mode=None,
        ohk=16, oh_bufs=3, act_build=0,
        trace=False, debug_skip_gemm=False, debug_skip_scatter=False,
        repeat=1):
    x = np.ascontiguousarray(np.asarray(inputs["x"], dtype=np.float32))
    weight = np.ascontiguousarray(np.asarray(inputs["weight"], dtype=np.float32))
    bias = np.ascontiguousarray(
        np.asarray(inputs["bias"], dtype=np.float32).reshape(dout, 1))
    meta = preprocess(
        inputs["edge_src"], inputs["edge_dst"], inputs["edge_vals"],
        n_nodes=n_nodes, cores=cores, win=win)
    shard = meta["shard"]

    nc = build_program(meta, n_nodes=n_nodes, din=din, dout=dout, cores=cores,
                       win=win, maxb=maxb, msgs_bufs=msgs_bufs,
                       gp_build=gp_build, sc_ps_bufs=sc_ps_bufs,
                       debug_mode=debug_mode, ohk=ohk, oh_bufs=oh_bufs,
                       act_build=act_build,
                       debug_skip_gemm=debug_skip_gemm,
                       debug_skip_scatter=debug_skip_scatter, repeat=repeat)

    xT = np.ascontiguousarray(x.T)
    in_maps = []
    for c in range(cores):
        m = {
            "xT": np.ascontiguousarray(xT[:, c * shard:(c + 1) * shard]),
            "w": weight,
            "bias": bias,
        }
        if meta["nlo"]:
            m["idx_lo"] = meta["idx_lo"][c]
            m["dst_lo"] = meta["dst_lo"][c]
            m["val_lo"] = meta["val_lo"][c]
        if meta["nhi"]:
            m["idx_hi"] = meta["idx_hi"][c]
            m["dst_hi"] = meta["dst_hi"][c]
            m["val_hi"] = meta["val_hi"][c]
        if debug_skip_gemm:
            m["xw_full"] = np.ascontiguousarray(x @ weight)
        in_maps.append(m)

    res = bass_utils.run_bass_kernel_spmd(
        nc, in_maps, core_ids=list(range(cores)), trace=trace)
    out = np.concatenate(
        [res.results[c]["outT"].T for c in range(cores)], axis=0)
    run.last_nc = nc
    run.last_in_maps = in_maps
    return out, res


def kernel(**inputs):
    out, _ = run(
        inputs,
        n_nodes=50000, n_edges=800000, din=256, dout=64, cores=8,
        maxb=48, msgs_bufs=4, ohk=0, oh_bufs=8,
    )
    return np.ascontiguousarray(out, dtype=np.float32)



# revision 17
# speedup vs baseline: 1.6401x; 1.6401x over previous
"""GCN layer (x @ W -> edge gather/scale/scatter-add -> +bias, relu) on 8 NeuronCores.

Measured: ~650 us/iteration on 8 axon-tunneled trn2 cores, rel err 4.5e-07
vs the fp32 jax reference (timing via on-device repeat slope; the axon RPC
adds ~95 ms/call that the slope cancels).

Strategy (per sharding hint):
  - Shard nodes across 8 cores (6250 rows each). Each core computes its local
    xw = x_shard @ W on the PE (host pre-transposes x so K lands on
    partitions), then an AllGather builds the full xw table [50000, 64] in
    every core's DRAM.
  - Edges are partitioned by destination shard on the host, sorted by
    destination window (128 nodes), grouped into 128-edge groups that each
    target a single window.  Per group the core:
      * dma_gather's the 128 source rows (256B each) from the xw table
        (SWDGE custom gather, 4 SWDGE queues round-robin -- queue depth is
        the gather throughput lever: 33 GB/s at q=1 vs 323 GB/s at q=4)
      * builds a one-hot*val matrix [128e, 128d] with one fused DVE
        tensor_scalar (iota == dstloc) * val   (ohk=0 path; ohk>0 selects a
        bulk tensor_tensor builder, measured slower in situ)
      * accumulates psum[64f, 128d] += msgs[128e, 64f].T @ onehot on the PE
        (messages are the 64-col stationary operand, onehot streams)
    Window accumulation ends with a fused bias+relu on the scalar engine
    (bias is per-partition in the transposed layout) into an output staging
    tile; one DMA stores outT [64, 6250] and the host transposes.
  - int16 gather indices only address <32768 rows, so edges are split into a
    "low" stream (src < 25000) and "high" stream (src >= 25000), gathered
    from the matching half of the xw table.  Group counts per (window, half)
    are padded to the max over cores (~+13% edges) so all 8 cores run one
    SPMD program (run_bass_kernel_spmd shares a single instruction stream).
  - Host reassembles: out = concat(outT_c.T).
"""

import os
import sys

import numpy as np


def _ensure_concourse():
    try:
        import concourse  # noqa: F401
        return
    except ImportError:
        pass
    for p in ("/opt/trn_rl_repo", "/root/.axon_site/_ro/trn_rl_repo"):
        if os.path.isdir(p):
            sys.path.insert(0, p)
            try:
                import concourse  # noqa: F401
                return
            except ImportError:
                sys.path.pop(0)
    raise ImportError("concourse (bass) not found")


_ensure_concourse()

import concourse.bacc as bacc  # noqa: E402
import concourse.mybir as mybir  # noqa: E402
import concourse.tile as tile  # noqa: E402
from concourse import bass_utils  # noqa: E402

F32 = mybir.dt.float32
BF16 = mybir.dt.bfloat16
I16 = mybir.dt.int16
I32 = mybir.dt.int32


def _cdiv(a, b):
    return -(-a // b)


def preprocess(edge_src, edge_dst, edge_vals, *, n_nodes, cores, win):
    """Partition/sort/pad edges into per-core low/high streams.

    Returns a dict with SPMD-uniform structure (G arrays) and per-core data
    arrays laid out exactly as the device consumes them.
    """
    shard = n_nodes // cores
    nwin = _cdiv(shard, win)
    half = n_nodes // 2

    src = np.asarray(edge_src).astype(np.int64)
    dst = np.asarray(edge_dst).astype(np.int64)
    vals = np.asarray(edge_vals).astype(np.float32)
    e = src.shape[0]

    core = dst // shard
    dl = dst - core * shard
    w = dl // win
    h = (src >= half).astype(np.int64)
    key = (core * nwin + w) * 2 + h

    order = np.argsort(key, kind="stable")
    ks = key[order]
    src_s = src[order]
    dloc_s = (dl - w * win)[order].astype(np.float32)
    v_s = vals[order]
    c_s = core[order]
    w_s = w[order]
    h_s = h[order]

    nbuck = cores * nwin * 2
    sizes = np.bincount(key, minlength=nbuck)
    starts = np.concatenate(([0], np.cumsum(sizes)))[:-1]
    rank = np.arange(e, dtype=np.int64) - starts[ks]

    # groups per (window, half): max over cores
    cnt = sizes.reshape(cores, nwin, 2)
    G = _cdiv(cnt, 128).max(axis=0)  # [nwin, 2]
    glo, ghi = G[:, 0], G[:, 1]
    cum_lo = np.concatenate(([0], np.cumsum(glo)))  # group offsets per window
    cum_hi = np.concatenate(([0], np.cumsum(ghi)))
    gtot_lo, gtot_hi = int(cum_lo[-1]), int(cum_hi[-1])
    nlo, nhi = gtot_lo * 128, gtot_hi * 128

    idx_lo = np.zeros((cores, max(nlo, 1)), np.int16)
    dst_lo = np.zeros((cores, max(nlo, 1)), np.float32)
    val_lo = np.zeros((cores, max(nlo, 1)), np.float32)
    idx_hi = np.zeros((cores, max(nhi, 1)), np.int16)
    dst_hi = np.zeros((cores, max(nhi, 1)), np.float32)
    val_hi = np.zeros((cores, max(nhi, 1)), np.float32)

    m = h_s == 0
    pos = cum_lo[w_s[m]] * 128 + rank[m]
    idx_lo[c_s[m], pos] = src_s[m].astype(np.int16)
    dst_lo[c_s[m], pos] = dloc_s[m]
    val_lo[c_s[m], pos] = v_s[m]

    m = h_s == 1
    pos = cum_hi[w_s[m]] * 128 + rank[m]
    idx_hi[c_s[m], pos] = (src_s[m] - half).astype(np.int16)
    dst_hi[c_s[m], pos] = dloc_s[m]
    val_hi[c_s[m], pos] = v_s[m]

    def idx_layout(a, n):
        # logical position i -> [i % 16, i // 16], replicated to 128 partitions
        if n == 0:
            return None
        blk = a[:n].reshape(-1, 16).T  # [16, n/16]
        return np.ascontiguousarray(np.tile(blk, (8, 1)))  # [128, n/16]

    def grp_layout(a, n):
        # position g*128+p -> [p, g]
        if n == 0:
            return None
        return np.ascontiguousarray(a[:n].reshape(-1, 128).T)  # [128, G]

    return dict(
        shard=shard,
        nwin=nwin,
        half=half,
        glo=glo,
        ghi=ghi,
        cum_lo=cum_lo,
        cum_hi=cum_hi,
        gtot_lo=gtot_lo,
        gtot_hi=gtot_hi,
        nlo=nlo,
        nhi=nhi,
        idx_lo=[idx_layout(idx_lo[c], nlo) for c in range(cores)],
        dst_lo=[grp_layout(dst_lo[c], nlo) for c in range(cores)],
        val_lo=[grp_layout(val_lo[c], nlo) for c in range(cores)],
        idx_hi=[idx_layout(idx_hi[c], nhi) for c in range(cores)],
        dst_hi=[grp_layout(dst_hi[c], nhi) for c in range(cores)],
        val_hi=[grp_layout(val_hi[c], nhi) for c in range(cores)],
    )


def build_program(meta, *, n_nodes, din, dout, cores, win, maxb=64,
                  msgs_bufs=2, gp_build=0, sc_ps_bufs=4, debug_mode=None,
                  ohk=16, oh_bufs=3, act_build=0,
                  debug_skip_gemm=False, debug_skip_scatter=False, repeat=1):
    """Build the SPMD Bass program. Returns (nc, input_names)."""
    shard = meta["shard"]
    nwin = meta["nwin"]
    half = meta["half"]
    glo, ghi = meta["glo"], meta["ghi"]
    cum_lo, cum_hi = meta["cum_lo"], meta["cum_hi"]
    gtot_lo, gtot_hi = meta["gtot_lo"], meta["gtot_hi"]
    nlo, nhi = meta["nlo"], meta["nhi"]
    kch = _cdiv(din, 128)
    nr = _cdiv(shard, 128)

    # padded bf16 table row: dout reals + pad to 256B so dma_gather's
    # 256B-stride restriction is met; gather traffic is unchanged vs fp32
    # but messages arrive bf16 (PE 4x, DVE 2x vs fp32).
    drow = 128
    assert dout <= drow

    nc = bacc.Bacc("TRN2", target_bir_lowering=False, debug=False,
                   num_devices=cores, num_swdge_queues=4)

    t_xT = nc.dram_tensor("xT", [din, shard], BF16, kind="ExternalInput")
    t_w = nc.dram_tensor("w", [din, dout], BF16, kind="ExternalInput")
    t_bias = nc.dram_tensor("bias", [dout, 1], F32, kind="ExternalInput")
    t_idx = {}
    t_dst = {}
    t_val = {}
    if nlo:
        t_idx["lo"] = nc.dram_tensor("idx_lo", [128, nlo // 16], I16, kind="ExternalInput")
        t_dst["lo"] = nc.dram_tensor("dst_lo", [128, gtot_lo], F32, kind="ExternalInput")
        t_val["lo"] = nc.dram_tensor("val_lo", [128, gtot_lo], F32, kind="ExternalInput")
    if nhi:
        t_idx["hi"] = nc.dram_tensor("idx_hi", [128, nhi // 16], I16, kind="ExternalInput")
        t_dst["hi"] = nc.dram_tensor("dst_hi", [128, gtot_hi], F32, kind="ExternalInput")
        t_val["hi"] = nc.dram_tensor("val_hi", [128, gtot_hi], F32, kind="ExternalInput")
    t_out = nc.dram_tensor("outT", [dout, shard], F32, kind="ExternalOutput")

    if debug_skip_gemm:
        t_xw_full = nc.dram_tensor("xw_full", [n_nodes, drow], BF16,
                                   kind="ExternalInput")
    else:
        t_xw_shard = nc.dram_tensor("xw_shard", [shard, drow], BF16)
        t_xw_full = nc.dram_tensor("xw_full", [n_nodes, drow], BF16,
                                   addr_space="Shared" if cores > 4 else "Local")

    with tile.TileContext(nc) as tc:
        with (
            tc.tile_pool(name="const", bufs=1) as constp,
            tc.tile_pool(name="xt", bufs=1) as xtp,
            tc.tile_pool(name="stage", bufs=1) as stagep,
            tc.tile_pool(name="meta", bufs=1) as metap,
            tc.tile_pool(name="msgs_lo", bufs=msgs_bufs) as mlp,
            tc.tile_pool(name="msgs_hi", bufs=msgs_bufs) as mhp,
            tc.tile_pool(name="oh", bufs=oh_bufs) as ohp,
            tc.tile_pool(name="gemm_ps", bufs=2, space="PSUM") as gpsp,
            tc.tile_pool(name="sc_ps", bufs=sc_ps_bufs, space="PSUM") as spsp,
        ):
            # ---- constants ----
            # iota repeated max(ohk,1) times: iota_rep[p, k*win + j] = j
            iw = max(ohk, 1)
            iota_i = constp.tile([128, iw * win], I32)
            nc.gpsimd.iota(iota_i[:], pattern=[[0, iw], [1, win]], base=0,
                           channel_multiplier=0)
            iota_f = constp.tile([128, iw * win], BF16)
            nc.vector.tensor_copy(iota_f[:], iota_i[:])
            bias_sb = constp.tile([dout, 1], F32)
            nc.sync.dma_start(bias_sb[:], t_bias[:])
            w_sb = constp.tile([128, kch * dout], BF16)
            nc.sync.dma_start(
                w_sb[:].rearrange("p (k e) -> p k e", e=dout),
                t_w[:].rearrange("(k p) e -> p k e", p=128),
            )

            # ---- metadata loads ----
            sb_idx = {}
            sb_dst = {}
            sb_val = {}
            for s in t_idx:
                sb_idx[s] = metap.tile(list(t_idx[s].shape), I16, tag=f"idx_{s}", name=f"sb_idx_{s}")
                nc.sync.dma_start(sb_idx[s][:], t_idx[s][:])
                sb_dst[s] = metap.tile(list(t_dst[s].shape), F32, tag=f"dst_{s}", name=f"sb_dst_{s}")
                nc.sync.dma_start(sb_dst[s][:], t_dst[s][:])
                sb_val[s] = metap.tile(list(t_val[s].shape), F32, tag=f"val_{s}", name=f"sb_val_{s}")
                nc.sync.dma_start(sb_val[s][:], t_val[s][:])
            sb_negd = {}
            sb_negv = {}
            if act_build:
                for s in t_idx:
                    sb_negd[s] = metap.tile(list(t_dst[s].shape), F32,
                                            tag=f"negd_{s}", name=f"sb_negd_{s}")
                    nc.vector.tensor_scalar(
                        sb_negd[s][:], sb_dst[s][:], -1.0, None,
                        mybir.AluOpType.mult)
                    sb_negv[s] = metap.tile(list(t_val[s].shape), F32,
                                            tag=f"negv_{s}", name=f"sb_negv_{s}")
                    nc.vector.tensor_scalar(
                        sb_negv[s][:], sb_val[s][:], -1.0, None,
                        mybir.AluOpType.mult)

            # ---- per-iteration body (repeat>1 used only for timing) ----
            def emit_body(rep):
                # ---- local GEMM: xw_shard = x_shard @ W ----
                if debug_skip_gemm:
                    xt_sb = None
                else:
                    xt_sb = []
                    for k in range(kch):
                        kp = min(128, din - k * 128)
                        xt = xtp.tile([kp, shard], BF16, tag=f"xt{k}")
                        nc.sync.dma_start(xt[:], t_xT[k * 128:k * 128 + kp, :])
                        xt_sb.append(xt)
                    xw_stage = stagep.tile([128, nr * drow], BF16, tag="xw_stage")
                    # pad cols are never read by the matmuls, but zero them so
                    # the gathered bytes are defined for the simulator
                    nc.vector.memset(
                        xw_stage[:].rearrange("p (r e) -> p r e", e=drow)[:, :, dout:],
                        0.0)
                    for r in range(nr):
                        rw = min(128, shard - r * 128)
                        ps = gpsp.tile([rw, dout], F32, tag="gemm_ps")
                        for k in range(kch):
                            nc.tensor.matmul(
                                ps[:],
                                xt_sb[k][:, r * 128:r * 128 + rw],
                                w_sb[:xt_sb[k].shape[0], k * dout:(k + 1) * dout],
                                start=(k == 0),
                                stop=(k == kch - 1),
                            )
                        nc.scalar.activation(
                        xw_stage[:rw, r * drow:r * drow + dout], ps[:],
                        mybir.ActivationFunctionType.Copy)
                    # store xw_shard (row-major, 256B padded rows) then AllGather
                    nfull = shard // 128
                    nc.sync.dma_start(
                        t_xw_shard[: nfull * 128, :].rearrange("(r p) e -> p r e", p=128),
                        xw_stage[:, : nfull * drow].rearrange("p (r e) -> p r e", e=drow),
                    )
                    if shard > nfull * 128:
                        rw = shard - nfull * 128
                        nc.sync.dma_start(
                            t_xw_shard[nfull * 128:, :],
                            xw_stage[:rw, nfull * drow:(nfull + 1) * drow],
                        )
                    nc.gpsimd.collective_compute(
                        "AllGather",
                        mybir.AluOpType.bypass,
                        replica_groups=[list(range(cores))],
                        ins=[t_xw_shard[:]],
                        outs=[t_xw_full[:]],
                    )

                # ---- scatter phase ----
                in_ap = {}
                if nlo:
                    in_ap["lo"] = t_xw_full[0:half, :]
                if nhi:
                    in_ap["hi"] = t_xw_full[half:n_nodes, :]
                gtot = {"lo": gtot_lo, "hi": gtot_hi}
                nbatch = {s: _cdiv(gtot[s], maxb) for s in in_ap}
                pool = {"lo": mlp, "hi": mhp}
                msgs_buf = {s: [None] * nbatch[s] for s in in_ap}
                qctr = [0]
                noh = {s: _cdiv(gtot[s], max(ohk, 1)) for s in in_ap}
                oh_buf = {s: [None] * noh[s] for s in in_ap}
                scaled = {s: [False] * nbatch[s] for s in in_ap}

                def emit_oh(s, c):
                    g0 = c * ohk
                    gn = min(ohk, gtot[s] - g0)
                    buf = ohp.tile([128, gn * win], BF16, tag="oh",
                                   name=f"oh_{s}_{c}_r{rep}")
                    nc.vector.tensor_tensor(
                        buf[:].rearrange("p (k j) -> p k j", j=win),
                        iota_f[:, :gn * win].rearrange("p (k j) -> p k j", j=win),
                        sb_dst[s][:, g0:g0 + gn].broadcast_to([128, gn, win]),
                        op=mybir.AluOpType.is_equal,
                    )
                    oh_buf[s][c] = buf

                def emit_gather(s, b):
                    g0 = b * maxb
                    gn = min(maxb, gtot[s] - g0)
                    n_idx = gn * 128
                    buf = pool[s].tile([128, gn * drow], BF16, tag=f"msgs_{s}", name=f"msgs_{s}_{b}_r{rep}")
                    nc.gpsimd.dma_gather(
                        buf[:].rearrange("p (c e) -> p c e", e=drow),
                        in_ap[s],
                        sb_idx[s][:, g0 * 8:(g0 + gn) * 8],
                        n_idx,
                        n_idx,
                        drow,
                        single_packet=False,
                        queue_num=qctr[0] % 4,
                    )
                    qctr[0] += 1
                    msgs_buf[s][b] = buf

                def emit_scale(s, b):
                    # fold edge values into the messages: one bulk multiply.
                    # Deferred to first consumption so the DVE doesn't
                    # head-of-line block on the gather DMA.
                    g0 = b * maxb
                    gn = min(maxb, gtot[s] - g0)
                    buf = msgs_buf[s][b]
                    nc.vector.tensor_tensor(
                        buf[:].rearrange("p (c e) -> p c e", e=drow)[:, :, :dout],
                        buf[:].rearrange("p (c e) -> p c e", e=drow)[:, :, :dout],
                        sb_val[s][:, g0:g0 + gn].broadcast_to([128, gn, dout]),
                        op=mybir.AluOpType.mult,
                    )
                    scaled[s][b] = True

                out_stage = stagep.tile([dout, shard], F32, tag="out_stage")
                cum = {"lo": cum_lo, "hi": cum_hi}
                if debug_skip_scatter == "gather_only":
                    # gathers only; consume each batch with one cheap DVE add
                    acc = stagep.tile([128, dout], BF16, tag="dbg_acc")
                    nc.vector.memset(acc[:], 0.0)
                    for s in in_ap:
                        for b in range(nbatch[s]):
                            emit_gather(s, b)
                            nc.vector.tensor_tensor(
                                acc[:], acc[:], msgs_buf[s][b][:, :dout],
                                op=mybir.AluOpType.add)
                    nc.vector.tensor_copy(out_stage[:, :dout], acc[:dout, :dout])
                    nc.sync.dma_start(t_out[:], out_stage[:])
                    return
                if debug_skip_scatter:
                    # debug: outT = xw_full[core-shard].T via strided DMA read
                    nc.sync.dma_start(
                        out_stage[:],
                        t_xw_full[0:shard, :].rearrange("n e -> e n"))
                    nc.sync.dma_start(t_out[:], out_stage[:])
                    nwin_eff = 0
                else:
                    nwin_eff = nwin
                for wi in range(nwin_eff):
                    ww = min(win, shard - wi * win)
                    spans = [(s, int(cum[s][wi]), int(cum[s][wi + 1])) for s in in_ap]
                    ngrp = sum(g1 - g0 for _, g0, g1 in spans)
                    if ngrp == 0:
                        # no edges anywhere for this window: bias + relu of zero
                        zps = spsp.tile([dout, win], F32, tag="sc_ps")
                        nc.vector.memset(zps[:], 0.0)
                        nc.scalar.activation(
                            out_stage[:, wi * win:wi * win + ww], zps[:, :ww],
                            mybir.ActivationFunctionType.Relu, bias=bias_sb[:],
                        )
                        continue
                    ps = spsp.tile([dout, win], F32, tag="sc_ps")
                    gi = 0
                    for s, g0, g1 in spans:
                        for g in range(g0, g1):
                            b, j = g // maxb, g % maxb
                            if msgs_buf[s][b] is None:
                                emit_gather(s, b)
                            if debug_mode == "const_oh":
                                oh_ap = iota_f[:, :win]
                            elif ohk == 0:
                                oh = ohp.tile([128, win], BF16, tag="oh")
                                if act_build and gi % act_build == act_build - 1:
                                    # scalar-engine build (exact for int iota):
                                    # oh = val * relu(1 - |iota - dst|)
                                    ab = ohp.tile([128, win], BF16, tag="abst")
                                    nc.scalar.activation(
                                        ab[:], iota_f[:, :win],
                                        mybir.ActivationFunctionType.Abs,
                                        bias=sb_negd[s][:, g:g + 1])
                                    nc.scalar.activation(
                                        oh[:], ab[:],
                                        mybir.ActivationFunctionType.Relu,
                                        bias=sb_val[s][:, g:g + 1],
                                        scale=sb_negv[s][:, g:g + 1])
                                else:
                                    # per-group fused build on DVE
                                    nc.vector.tensor_scalar(
                                        oh[:],
                                        iota_f[:, :win],
                                        sb_dst[s][:, g:g + 1],
                                        sb_val[s][:, g:g + 1],
                                        mybir.AluOpType.is_equal,
                                        mybir.AluOpType.mult,
                                    )
                                oh_ap = oh[:]
                            else:
                                if not scaled[s][b]:
                                    emit_scale(s, b)
                                oc, ojj = g // ohk, g % ohk
                                if oh_buf[s][oc] is None:
                                    emit_oh(s, oc)
                                oh_ap = oh_buf[s][oc][:, ojj * win:(ojj + 1) * win]
                            if debug_mode == "no_mm":
                                if gi == 0:
                                    nc.tensor.matmul(
                                        ps[:], msgs_buf[s][b][:, j * drow:j * drow + dout],
                                        oh_ap, start=True, stop=True)
                            else:
                                nc.tensor.matmul(
                                    ps[:],
                                    msgs_buf[s][b][:, j * drow:j * drow + dout],
                                    oh_ap,
                                    start=(gi == 0),
                                    stop=(gi == ngrp - 1),
                                )
                            gi += 1
                    nc.scalar.activation(
                        out_stage[:, wi * win:wi * win + ww], ps[:, :ww],
                        mybir.ActivationFunctionType.Relu, bias=bias_sb[:],
                    )
                nc.sync.dma_start(t_out[:], out_stage[:])

            for _rep in range(repeat):
                emit_body(_rep)

    nc.compile()
    return nc


def run(inputs, *, n_nodes, n_edges, din, dout, cores, win=128, maxb=64,
        msgs_bufs=2, gp_build=0, sc_ps_bufs=4, debug_mode=None,
        ohk=16, oh_bufs=3, act_build=0,
        trace=False, debug_skip_gemm=False, debug_skip_scatter=False,
        repeat=1):
    import ml_dtypes

    x = np.ascontiguousarray(np.asarray(inputs["x"], dtype=np.float32))
    weight = np.ascontiguousarray(np.asarray(inputs["weight"], dtype=np.float32))
    bias = np.ascontiguousarray(
        np.asarray(inputs["bias"], dtype=np.float32).reshape(dout, 1))
    meta = preprocess(
        inputs["edge_src"], inputs["edge_dst"], inputs["edge_vals"],
        n_nodes=n_nodes, cores=cores, win=win)
    shard = meta["shard"]

    nc = build_program(meta, n_nodes=n_nodes, din=din, dout=dout, cores=cores,
                       win=win, maxb=maxb, msgs_bufs=msgs_bufs,
                       gp_build=gp_build, sc_ps_bufs=sc_ps_bufs,
                       debug_mode=debug_mode, ohk=ohk, oh_bufs=oh_bufs,
                       act_build=act_build,
                       debug_skip_gemm=debug_skip_gemm,
                       debug_skip_scatter=debug_skip_scatter, repeat=repeat)

    xT = np.ascontiguousarray(x.T.astype(ml_dtypes.bfloat16))
    w_bf = np.ascontiguousarray(weight.astype(ml_dtypes.bfloat16))
    in_maps = []
    for c in range(cores):
        m = {
            "xT": np.ascontiguousarray(xT[:, c * shard:(c + 1) * shard]),
            "w": w_bf,
            "bias": bias,
        }
        if meta["nlo"]:
            m["idx_lo"] = meta["idx_lo"][c]
            m["dst_lo"] = meta["dst_lo"][c]
            m["val_lo"] = meta["val_lo"][c]
        if meta["nhi"]:
            m["idx_hi"] = meta["idx_hi"][c]
            m["dst_hi"] = meta["dst_hi"][c]
            m["val_hi"] = meta["val_hi"][c]
        if debug_skip_gemm:
            xw = (x @ weight).astype(ml_dtypes.bfloat16)
            xw_pad = np.zeros((n_nodes, 128), ml_dtypes.bfloat16)
            xw_pad[:, :dout] = xw
            m["xw_full"] = xw_pad
        in_maps.append(m)

    res = bass_utils.run_bass_kernel_spmd(
        nc, in_maps, core_ids=list(range(cores)), trace=trace)
    out = np.concatenate(
        [res.results[c]["outT"].T for c in range(cores)], axis=0)
    run.last_nc = nc
    run.last_in_maps = in_maps
    return out, res


def kernel(**inputs):
    out, _ = run(
        inputs,
        n_nodes=50000, n_edges=800000, din=256, dout=64, cores=8,
        maxb=48, msgs_bufs=4, ohk=0, oh_bufs=8,
    )
    return np.ascontiguousarray(out, dtype=np.float32)



# revision 42
# speedup vs baseline: 2.2245x; 1.3563x over previous
"""GCN layer (x @ W -> edge gather/scale/scatter-add -> +bias, relu) on 8 NeuronCores.

Measured: ~650 us/iteration on 8 axon-tunneled trn2 cores, rel err 4.5e-07
vs the fp32 jax reference (timing via on-device repeat slope; the axon RPC
adds ~95 ms/call that the slope cancels).

Strategy (per sharding hint):
  - Shard nodes across 8 cores (6250 rows each). Each core computes its local
    xw = x_shard @ W on the PE (host pre-transposes x so K lands on
    partitions), then an AllGather builds the full xw table [50000, 64] in
    every core's DRAM.
  - Edges are partitioned by destination shard on the host, sorted by
    destination window (128 nodes), grouped into 128-edge groups that each
    target a single window.  Per group the core:
      * dma_gather's the 128 source rows (256B each) from the xw table
        (SWDGE custom gather, 4 SWDGE queues round-robin -- queue depth is
        the gather throughput lever: 33 GB/s at q=1 vs 323 GB/s at q=4)
      * builds a one-hot*val matrix [128e, 128d] with one fused DVE
        tensor_scalar (iota == dstloc) * val   (ohk=0 path; ohk>0 selects a
        bulk tensor_tensor builder, measured slower in situ)
      * accumulates psum[64f, 128d] += msgs[128e, 64f].T @ onehot on the PE
        (messages are the 64-col stationary operand, onehot streams)
    Window accumulation ends with a fused bias+relu on the scalar engine
    (bias is per-partition in the transposed layout) into an output staging
    tile; one DMA stores outT [64, 6250] and the host transposes.
  - int16 gather indices only address <32768 rows, so edges are split into a
    "low" stream (src < 25000) and "high" stream (src >= 25000), gathered
    from the matching half of the xw table.  Group counts per (window, half)
    are padded to the max over cores (~+13% edges) so all 8 cores run one
    SPMD program (run_bass_kernel_spmd shares a single instruction stream).
  - Host reassembles: out = concat(outT_c.T).
"""

import os
import sys

import numpy as np


def _ensure_concourse():
    try:
        import concourse  # noqa: F401
        return
    except ImportError:
        pass
    for p in ("/opt/trn_rl_repo", "/root/.axon_site/_ro/trn_rl_repo"):
        if os.path.isdir(p):
            sys.path.insert(0, p)
            try:
                import concourse  # noqa: F401
                return
            except ImportError:
                sys.path.pop(0)
    raise ImportError("concourse (bass) not found")


_ensure_concourse()

import concourse.bacc as bacc  # noqa: E402
import concourse.mybir as mybir  # noqa: E402
import concourse.tile as tile  # noqa: E402
from concourse import bass_utils  # noqa: E402

F32 = mybir.dt.float32
BF16 = mybir.dt.bfloat16
I16 = mybir.dt.int16
I32 = mybir.dt.int32


def _cdiv(a, b):
    return -(-a // b)


def preprocess(edge_src, edge_dst, edge_vals, *, n_nodes, cores, win,
               src_sort=True):
    """Partition/sort/pad edges into per-core low/high streams.

    Returns a dict with SPMD-uniform structure (G arrays) and per-core data
    arrays laid out exactly as the device consumes them.
    """
    shard = n_nodes // cores
    nwin = _cdiv(shard, win)
    half = n_nodes // 2
    hs = shard // 2

    src = np.asarray(edge_src).astype(np.int64)
    dst = np.asarray(edge_dst).astype(np.int64)
    vals = np.asarray(edge_vals).astype(np.float32)
    e = src.shape[0]

    core = dst // shard
    dl = dst - core * shard
    w = dl // win
    # lo/hi = first/second half of each source core's shard rows; the table
    # is laid out chunk-major (all cores' first halves, then second halves)
    # so the AllGather can be split into two chunks and the lo-stream scatter
    # overlapped with the second chunk.
    sc = src // shard
    sr = src - sc * shard
    h = (sr >= hs).astype(np.int64)
    srcidx = sc * hs + (sr - h * hs)
    key = (core * nwin + w) * 2 + h

    assert shard % 2 == 0
    if src_sort:
        # ascending src within each bucket: the gather's 256B random reads
        # then sweep HBM addresses monotonically per bucket
        order = np.lexsort((srcidx, key))
    else:
        order = np.argsort(key, kind="stable")
    ks = key[order]
    src_s = srcidx[order]
    dloc_s = (dl - w * win)[order].astype(np.float32)
    v_s = vals[order]
    c_s = core[order]
    w_s = w[order]
    h_s = h[order]

    nbuck = cores * nwin * 2
    sizes = np.bincount(key, minlength=nbuck)
    starts = np.concatenate(([0], np.cumsum(sizes)))[:-1]
    rank = np.arange(e, dtype=np.int64) - starts[ks]

    # groups per (window, half): max over cores
    cnt = sizes.reshape(cores, nwin, 2)
    G = _cdiv(cnt, 128).max(axis=0)  # [nwin, 2]
    glo, ghi = G[:, 0], G[:, 1]
    cum_lo = np.concatenate(([0], np.cumsum(glo)))  # group offsets per window
    cum_hi = np.concatenate(([0], np.cumsum(ghi)))
    gtot_lo, gtot_hi = int(cum_lo[-1]), int(cum_hi[-1])
    nlo, nhi = gtot_lo * 128, gtot_hi * 128

    idx_lo = np.zeros((cores, max(nlo, 1)), np.int16)
    dst_lo = np.zeros((cores, max(nlo, 1)), np.float32)
    val_lo = np.zeros((cores, max(nlo, 1)), np.float32)
    idx_hi = np.zeros((cores, max(nhi, 1)), np.int16)
    dst_hi = np.zeros((cores, max(nhi, 1)), np.float32)
    val_hi = np.zeros((cores, max(nhi, 1)), np.float32)

    m = h_s == 0
    pos = cum_lo[w_s[m]] * 128 + rank[m]
    idx_lo[c_s[m], pos] = src_s[m].astype(np.int16)
    dst_lo[c_s[m], pos] = dloc_s[m]
    val_lo[c_s[m], pos] = v_s[m]

    m = h_s == 1
    pos = cum_hi[w_s[m]] * 128 + rank[m]
    idx_hi[c_s[m], pos] = src_s[m].astype(np.int16)
    dst_hi[c_s[m], pos] = dloc_s[m]
    val_hi[c_s[m], pos] = v_s[m]

    def idx_layout(a, n):
        # logical position i -> [i % 16, i // 16], replicated to 128 partitions
        if n == 0:
            return None
        blk = a[:n].reshape(-1, 16).T  # [16, n/16]
        return np.ascontiguousarray(np.tile(blk, (8, 1)))  # [128, n/16]

    def grp_layout(a, n):
        # position g*128+p -> [p, g]
        if n == 0:
            return None
        return np.ascontiguousarray(a[:n].reshape(-1, 128).T)  # [128, G]

    return dict(
        shard=shard,
        nwin=nwin,
        half=half,
        glo=glo,
        ghi=ghi,
        cum_lo=cum_lo,
        cum_hi=cum_hi,
        gtot_lo=gtot_lo,
        gtot_hi=gtot_hi,
        nlo=nlo,
        nhi=nhi,
        idx_lo=[idx_layout(idx_lo[c], nlo) for c in range(cores)],
        dst_lo=[grp_layout(dst_lo[c], nlo) for c in range(cores)],
        val_lo=[grp_layout(val_lo[c], nlo) for c in range(cores)],
        idx_hi=[idx_layout(idx_hi[c], nhi) for c in range(cores)],
        dst_hi=[grp_layout(dst_hi[c], nhi) for c in range(cores)],
        val_hi=[grp_layout(val_hi[c], nhi) for c in range(cores)],
    )


def build_program(meta, *, n_nodes, din, dout, cores, win, maxb=64,
                  msgs_bufs=2, gp_build=0, sc_ps_bufs=4, debug_mode=None,
                  ohk=16, oh_bufs=3, act_build=0, single_packet=False,
                  debug_skip_gemm=False, debug_skip_scatter=False, repeat=1):
    """Build the SPMD Bass program. Returns (nc, input_names)."""
    shard = meta["shard"]
    nwin = meta["nwin"]
    half = meta["half"]
    glo, ghi = meta["glo"], meta["ghi"]
    cum_lo, cum_hi = meta["cum_lo"], meta["cum_hi"]
    gtot_lo, gtot_hi = meta["gtot_lo"], meta["gtot_hi"]
    nlo, nhi = meta["nlo"], meta["nhi"]
    kch = _cdiv(din, 128)
    nr = _cdiv(shard, 128)

    # padded bf16 table row: dout reals + pad to 256B so dma_gather's
    # 256B-stride restriction is met; gather traffic is unchanged vs fp32
    # but messages arrive bf16 (PE 4x, DVE 2x vs fp32).
    drow = 128
    assert dout <= drow

    nc = bacc.Bacc("TRN2", target_bir_lowering=False, debug=False,
                   num_devices=cores, num_swdge_queues=4)

    t_xT = nc.dram_tensor("xT", [din, shard], BF16, kind="ExternalInput")
    t_w = nc.dram_tensor("w", [din, dout], BF16, kind="ExternalInput")
    t_bias = nc.dram_tensor("bias", [dout, 1], F32, kind="ExternalInput")
    t_idx = {}
    t_dst = {}
    t_val = {}
    if nlo:
        t_idx["lo"] = nc.dram_tensor("idx_lo", [128, nlo // 16], I16, kind="ExternalInput")
        t_dst["lo"] = nc.dram_tensor("dst_lo", [128, gtot_lo], F32, kind="ExternalInput")
        t_val["lo"] = nc.dram_tensor("val_lo", [128, gtot_lo], F32, kind="ExternalInput")
    if nhi:
        t_idx["hi"] = nc.dram_tensor("idx_hi", [128, nhi // 16], I16, kind="ExternalInput")
        t_dst["hi"] = nc.dram_tensor("dst_hi", [128, gtot_hi], F32, kind="ExternalInput")
        t_val["hi"] = nc.dram_tensor("val_hi", [128, gtot_hi], F32, kind="ExternalInput")
    t_out = nc.dram_tensor("outT", [dout, shard], F32, kind="ExternalOutput")

    if debug_skip_gemm == "shared":
        # probe: gather from a Shared-scratchpad tensor filled by DMA copy
        t_xw_in = nc.dram_tensor("xw_full", [n_nodes, drow], BF16,
                                 kind="ExternalInput")
        t_xw_full = nc.dram_tensor("xw_full_sh", [n_nodes, drow], BF16,
                                   addr_space="Shared")
    elif debug_skip_gemm:
        t_xw_full = nc.dram_tensor("xw_full", [n_nodes, drow], BF16,
                                   kind="ExternalInput")
    else:
        # double-buffered by rep parity so iteration k+1's GEMM+AllGather
        # overlaps iteration k's scatter (no WAR serialization between
        # back-to-back invocations)
        nbuf = min(repeat, 2)
        t_xw_shard = [nc.dram_tensor(f"xw_shard{i}", [shard, drow], BF16)
                      for i in range(nbuf)]
        t_xw_full = [nc.dram_tensor(f"xw_full{i}", [n_nodes, drow], BF16,
                                    addr_space="Shared" if cores > 4 else "Local")
                     for i in range(nbuf)]

    with tile.TileContext(nc) as tc:
        with (
            tc.tile_pool(name="const", bufs=1) as constp,
            tc.tile_pool(name="xt", bufs=1) as xtp,
            tc.tile_pool(name="stage", bufs=1) as stagep,
            tc.tile_pool(name="meta", bufs=1) as metap,
            tc.tile_pool(name="msgs_lo", bufs=msgs_bufs) as mlp,
            tc.tile_pool(name="msgs_hi", bufs=msgs_bufs) as mhp,
            tc.tile_pool(name="oh", bufs=oh_bufs) as ohp,
            tc.tile_pool(name="gemm_ps", bufs=2, space="PSUM") as gpsp,
            tc.tile_pool(name="sc_ps", bufs=sc_ps_bufs, space="PSUM") as spsp,
        ):
            # ---- constants ----
            # iota repeated max(ohk,1) times: iota_rep[p, k*win + j] = j
            iw = max(ohk, 1)
            iota_i = constp.tile([128, iw * win], I32)
            nc.gpsimd.iota(iota_i[:], pattern=[[0, iw], [1, win]], base=0,
                           channel_multiplier=0)
            iota_f = constp.tile([128, iw * win], BF16)
            nc.vector.tensor_copy(iota_f[:], iota_i[:])
            bias_sb = constp.tile([dout, 1], F32)
            nc.sync.dma_start(bias_sb[:], t_bias[:])
            w_sb = constp.tile([128, kch * dout], BF16)
            nc.sync.dma_start(
                w_sb[:].rearrange("p (k e) -> p k e", e=dout),
                t_w[:].rearrange("(k p) e -> p k e", p=128),
            )

            # ---- metadata loads ----
            sb_idx = {}
            sb_dst = {}
            sb_val = {}
            sb_dst16 = {}
            sb_val16 = {}
            for s in t_idx:
                sb_idx[s] = metap.tile(list(t_idx[s].shape), I16, tag=f"idx_{s}", name=f"sb_idx_{s}")
                nc.sync.dma_start(sb_idx[s][:], t_idx[s][:])
                sb_dst[s] = metap.tile(list(t_dst[s].shape), F32, tag=f"dst_{s}", name=f"sb_dst_{s}")
                nc.sync.dma_start(sb_dst[s][:], t_dst[s][:])
                sb_val[s] = metap.tile(list(t_val[s].shape), F32, tag=f"val_{s}", name=f"sb_val_{s}")
                nc.sync.dma_start(sb_val[s][:], t_val[s][:])
                if ohk:
                    sb_dst16[s] = metap.tile(list(t_dst[s].shape), BF16,
                                             tag=f"dst16_{s}", name=f"sb_dst16_{s}")
                    nc.vector.tensor_copy(sb_dst16[s][:], sb_dst[s][:])
                    sb_val16[s] = metap.tile(list(t_val[s].shape), BF16,
                                             tag=f"val16_{s}", name=f"sb_val16_{s}")
                    nc.vector.tensor_copy(sb_val16[s][:], sb_val[s][:])
            sb_negd = {}
            sb_negv = {}
            if act_build:
                for s in t_idx:
                    sb_negd[s] = metap.tile(list(t_dst[s].shape), F32,
                                            tag=f"negd_{s}", name=f"sb_negd_{s}")
                    nc.vector.tensor_scalar(
                        sb_negd[s][:], sb_dst[s][:], -1.0, None,
                        mybir.AluOpType.mult)
                    sb_negv[s] = metap.tile(list(t_val[s].shape), F32,
                                            tag=f"negv_{s}", name=f"sb_negv_{s}")
                    nc.vector.tensor_scalar(
                        sb_negv[s][:], sb_val[s][:], -1.0, None,
                        mybir.AluOpType.mult)

            # ---- per-iteration body (repeat>1 used only for timing) ----
            def emit_body(rep):
                # ---- local GEMM: xw_shard = x_shard @ W ----
                if debug_skip_gemm == "shared":
                    xt_sb = None
                    r_xw_full = t_xw_full
                    nc.sync.dma_start(t_xw_full[:], t_xw_in[:])
                elif debug_skip_gemm:
                    xt_sb = None
                    r_xw_full = t_xw_full
                else:
                    r_xw_shard = t_xw_shard[rep % len(t_xw_shard)]
                    r_xw_full = t_xw_full[rep % len(t_xw_full)]
                    xt_sb = []
                    for k in range(kch):
                        kp = min(128, din - k * 128)
                        xt = xtp.tile([kp, shard], BF16, tag=f"xt{k}")
                        nc.sync.dma_start(xt[:], t_xT[k * 128:k * 128 + kp, :])
                        xt_sb.append(xt)
                    xw_stage = stagep.tile([128, nr * drow], BF16, tag="xw_stage")
                    # pad cols are never read by the matmuls, but zero them so
                    # the gathered bytes are defined for the simulator
                    nc.vector.memset(
                        xw_stage[:].rearrange("p (r e) -> p r e", e=drow)[:, :, dout:],
                        0.0)
                    for r in range(nr):
                        rw = min(128, shard - r * 128)
                        ps = gpsp.tile([rw, dout], F32, tag="gemm_ps")
                        for k in range(kch):
                            nc.tensor.matmul(
                                ps[:],
                                xt_sb[k][:, r * 128:r * 128 + rw],
                                w_sb[:xt_sb[k].shape[0], k * dout:(k + 1) * dout],
                                start=(k == 0),
                                stop=(k == kch - 1),
                            )
                        nc.scalar.activation(
                        xw_stage[:rw, r * drow:r * drow + dout], ps[:],
                        mybir.ActivationFunctionType.Copy)
                    # store xw_shard (row-major, 256B padded rows) then AllGather
                    nfull = shard // 128
                    nc.sync.dma_start(
                        r_xw_shard[: nfull * 128, :].rearrange("(r p) e -> p r e", p=128),
                        xw_stage[:, : nfull * drow].rearrange("p (r e) -> p r e", e=drow),
                    )
                    if shard > nfull * 128:
                        rw = shard - nfull * 128
                        nc.sync.dma_start(
                            r_xw_shard[nfull * 128:, :],
                            xw_stage[:rw, nfull * drow:(nfull + 1) * drow],
                        )
                    # chunked AllGather: the lo-stream scatter only needs the
                    # first chunk, so its gathers overlap the second chunk
                    hs = shard // 2
                    nc.gpsimd.collective_compute(
                        "AllGather",
                        mybir.AluOpType.bypass,
                        replica_groups=[list(range(cores))],
                        ins=[r_xw_shard[0:hs, :]],
                        outs=[r_xw_full[0:cores * hs, :]],
                    )
                    nc.gpsimd.collective_compute(
                        "AllGather",
                        mybir.AluOpType.bypass,
                        replica_groups=[list(range(cores))],
                        ins=[r_xw_shard[hs:shard, :]],
                        outs=[r_xw_full[cores * hs:n_nodes, :]],
                    )

                # ---- scatter phase ----
                in_ap = {}
                if nlo:
                    in_ap["lo"] = r_xw_full[0:half, :]
                if nhi:
                    in_ap["hi"] = r_xw_full[half:n_nodes, :]
                gtot = {"lo": gtot_lo, "hi": gtot_hi}
                nbatch = {s: _cdiv(gtot[s], maxb) for s in in_ap}
                pool = {"lo": mlp, "hi": mhp}
                msgs_buf = {s: [None] * nbatch[s] for s in in_ap}
                qctr = [0]
                noh = {s: _cdiv(gtot[s], max(ohk, 1)) for s in in_ap}
                oh_buf = {s: [None] * noh[s] for s in in_ap}
                scaled = {s: [False] * nbatch[s] for s in in_ap}

                def emit_oh(s, c):
                    g0 = c * ohk
                    gn = min(ohk, gtot[s] - g0)
                    buf = ohp.tile([128, gn * win], BF16, tag="oh",
                                   name=f"oh_{s}_{c}_r{rep}")
                    nc.vector.tensor_tensor(
                        buf[:].rearrange("p (k j) -> p k j", j=win),
                        iota_f[:, :gn * win].rearrange("p (k j) -> p k j", j=win),
                        sb_dst16[s][:, g0:g0 + gn].broadcast_to([128, gn, win]),
                        op=mybir.AluOpType.is_equal,
                    )
                    oh_buf[s][c] = buf

                def emit_gather(s, b):
                    g0 = b * maxb
                    gn = min(maxb, gtot[s] - g0)
                    n_idx = gn * 128
                    buf = pool[s].tile([128, gn * drow], BF16, tag=f"msgs_{s}", name=f"msgs_{s}_{b}_r{rep}")
                    nc.gpsimd.dma_gather(
                        buf[:].rearrange("p (c e) -> p c e", e=drow),
                        in_ap[s],
                        sb_idx[s][:, g0 * 8:(g0 + gn) * 8],
                        n_idx,
                        n_idx,
                        drow,
                        single_packet=single_packet,
                        queue_num=qctr[0] % 4,
                    )
                    qctr[0] += 1
                    msgs_buf[s][b] = buf

                def emit_scale(s, b):
                    # fold edge values into the messages: one bulk multiply.
                    # Deferred to first consumption so the DVE doesn't
                    # head-of-line block on the gather DMA.
                    g0 = b * maxb
                    gn = min(maxb, gtot[s] - g0)
                    buf = msgs_buf[s][b]
                    nc.vector.tensor_tensor(
                        buf[:].rearrange("p (c e) -> p c e", e=drow)[:, :, :dout],
                        buf[:].rearrange("p (c e) -> p c e", e=drow)[:, :, :dout],
                        sb_val16[s][:, g0:g0 + gn].broadcast_to([128, gn, dout]),
                        op=mybir.AluOpType.mult,
                    )
                    scaled[s][b] = True

                out_stage = stagep.tile([dout, shard], F32, tag="out_stage")
                cum = {"lo": cum_lo, "hi": cum_hi}
                if debug_skip_scatter == "gather_only":
                    # gathers only; consume each batch with one cheap DVE add
                    acc = stagep.tile([128, dout], BF16, tag="dbg_acc")
                    nc.vector.memset(acc[:], 0.0)
                    for s in in_ap:
                        for b in range(nbatch[s]):
                            emit_gather(s, b)
                            nc.vector.tensor_tensor(
                                acc[:], acc[:], msgs_buf[s][b][:, :dout],
                                op=mybir.AluOpType.add)
                    nc.vector.tensor_copy(out_stage[:, :dout], acc[:dout, :dout])
                    nc.sync.dma_start(t_out[:], out_stage[:])
                    return
                if debug_skip_scatter:
                    # debug: consume a slice of xw_full so the collective is
                    # on the critical path; output is meaningless
                    nrows = (shard // 128) * 128
                    dbg = stagep.tile([128, nrows], BF16, tag="dbg_stage")
                    nc.sync.dma_start(
                        dbg[:].rearrange("p (r e) -> p r e", e=drow),
                        r_xw_full[0:nrows, :].rearrange("(r p) e -> p r e", p=128))
                    nc.vector.tensor_copy(
                        out_stage[:, :nrows], dbg[:dout, :nrows])
                    nc.sync.dma_start(t_out[:], out_stage[:])
                    nwin_eff = 0
                else:
                    nwin_eff = nwin
                def emit_chain(s, g0, g1, ps, gi0=0, ngrp=None):
                    if ngrp is None:
                        ngrp = g1 - g0
                    for gi, g in enumerate(range(g0, g1), start=gi0):
                        b, j = g // maxb, g % maxb
                        if msgs_buf[s][b] is None:
                            emit_gather(s, b)
                        if debug_mode == "const_oh":
                            oh_ap = iota_f[:, :win]
                        elif ohk == 0:
                            oh = ohp.tile([128, win], BF16, tag="oh")
                            if act_build and gi % act_build == act_build - 1:
                                # scalar-engine build (exact for int iota):
                                # oh = val * relu(1 - |iota - dst|)
                                ab = ohp.tile([128, win], BF16, tag="abst")
                                nc.scalar.activation(
                                    ab[:], iota_f[:, :win],
                                    mybir.ActivationFunctionType.Abs,
                                    bias=sb_negd[s][:, g:g + 1])
                                nc.scalar.activation(
                                    oh[:], ab[:],
                                    mybir.ActivationFunctionType.Relu,
                                    bias=sb_val[s][:, g:g + 1],
                                    scale=sb_negv[s][:, g:g + 1])
                            else:
                                # per-group fused build on DVE
                                nc.vector.tensor_scalar(
                                    oh[:],
                                    iota_f[:, :win],
                                    sb_dst[s][:, g:g + 1],
                                    sb_val[s][:, g:g + 1],
                                    mybir.AluOpType.is_equal,
                                    mybir.AluOpType.mult,
                                )
                            oh_ap = oh[:]
                        else:
                            if not scaled[s][b]:
                                emit_scale(s, b)
                            oc, ojj = g // ohk, g % ohk
                            if oh_buf[s][oc] is None:
                                emit_oh(s, oc)
                            oh_ap = oh_buf[s][oc][:, ojj * win:(ojj + 1) * win]
                        if debug_mode == "no_mm":
                            if gi == 0:
                                nc.tensor.matmul(
                                    ps[:], msgs_buf[s][b][:, j * drow:j * drow + dout],
                                    oh_ap, start=True, stop=True)
                        else:
                            nc.tensor.matmul(
                                ps[:],
                                msgs_buf[s][b][:, j * drow:j * drow + dout],
                                oh_ap,
                                start=(gi == 0),
                                stop=(gi == ngrp - 1),
                            )

                def wspan(s, wi):
                    if s not in in_ap:
                        return 0, 0
                    return int(cum[s][wi]), int(cum[s][wi + 1])

                for wi in range(nwin_eff):
                    ww = min(win, shard - wi * win)
                    sl = slice(wi * win, wi * win + ww)
                    spans = [(s, *wspan(s, wi)) for s in in_ap]
                    ngrp = sum(g1 - g0 for _, g0, g1 in spans)
                    if ngrp == 0:
                        # no edges anywhere for this window: bias + relu of zero
                        zps = spsp.tile([dout, win], F32, tag="sc_ps")
                        nc.vector.memset(zps[:], 0.0)
                        nc.scalar.activation(
                            out_stage[:, sl], zps[:, :ww],
                            mybir.ActivationFunctionType.Relu, bias=bias_sb[:],
                        )
                        continue
                    ps = spsp.tile([dout, win], F32, tag="sc_ps")
                    gi = 0
                    for s, g0, g1 in spans:
                        if g1 > g0:
                            emit_chain(s, g0, g1, ps, gi, ngrp)
                            gi += g1 - g0
                    nc.scalar.activation(
                        out_stage[:, sl], ps[:, :ww],
                        mybir.ActivationFunctionType.Relu, bias=bias_sb[:],
                    )
                nc.sync.dma_start(t_out[:], out_stage[:])

            for _rep in range(repeat):
                emit_body(_rep)

    nc.compile()
    return nc


def run(inputs, *, n_nodes, n_edges, din, dout, cores, win=128, maxb=64,
        msgs_bufs=2, gp_build=0, sc_ps_bufs=4, debug_mode=None,
        ohk=16, oh_bufs=3, act_build=0, single_packet=False, src_sort=True,
        trace=False, debug_skip_gemm=False, debug_skip_scatter=False,
        repeat=1):
    import ml_dtypes

    x = np.ascontiguousarray(np.asarray(inputs["x"], dtype=np.float32))
    weight = np.ascontiguousarray(np.asarray(inputs["weight"], dtype=np.float32))
    bias = np.ascontiguousarray(
        np.asarray(inputs["bias"], dtype=np.float32).reshape(dout, 1))
    meta = preprocess(
        inputs["edge_src"], inputs["edge_dst"], inputs["edge_vals"],
        n_nodes=n_nodes, cores=cores, win=win, src_sort=src_sort)
    shard = meta["shard"]

    nc = build_program(meta, n_nodes=n_nodes, din=din, dout=dout, cores=cores,
                       win=win, maxb=maxb, msgs_bufs=msgs_bufs,
                       gp_build=gp_build, sc_ps_bufs=sc_ps_bufs,
                       debug_mode=debug_mode, ohk=ohk, oh_bufs=oh_bufs,
                       act_build=act_build, single_packet=single_packet,
                       debug_skip_gemm=debug_skip_gemm,
                       debug_skip_scatter=debug_skip_scatter, repeat=repeat)

    xT = np.ascontiguousarray(x.T.astype(ml_dtypes.bfloat16))
    w_bf = np.ascontiguousarray(weight.astype(ml_dtypes.bfloat16))
    in_maps = []
    for c in range(cores):
        m = {
            "xT": np.ascontiguousarray(xT[:, c * shard:(c + 1) * shard]),
            "w": w_bf,
            "bias": bias,
        }
        if meta["nlo"]:
            m["idx_lo"] = meta["idx_lo"][c]
            m["dst_lo"] = meta["dst_lo"][c]
            m["val_lo"] = meta["val_lo"][c]
        if meta["nhi"]:
            m["idx_hi"] = meta["idx_hi"][c]
            m["dst_hi"] = meta["dst_hi"][c]
            m["val_hi"] = meta["val_hi"][c]
        if debug_skip_gemm:
            xw = (x @ weight).astype(ml_dtypes.bfloat16)
            xw_pad = np.zeros((n_nodes, 128), ml_dtypes.bfloat16)
            xw_pad[:, :dout] = xw
            m["xw_full"] = xw_pad
        in_maps.append(m)

    res = bass_utils.run_bass_kernel_spmd(
        nc, in_maps, core_ids=list(range(cores)), trace=trace)
    out = np.concatenate(
        [res.results[c]["outT"].T for c in range(cores)], axis=0)
    run.last_nc = nc
    run.last_in_maps = in_maps
    return out, res


def kernel(**inputs):
    out, _ = run(
        inputs,
        n_nodes=50000, n_edges=800000, din=256, dout=64, cores=8,
        maxb=32, msgs_bufs=5, ohk=16, oh_bufs=3,
    )
    return np.ascontiguousarray(out, dtype=np.float32)



# revision 43
# speedup vs baseline: 2.2531x; 1.0129x over previous
"""GCN layer (x @ W -> edge gather/scale/scatter-add -> +bias, relu) on 8 NeuronCores.

Measured: ~377 us/iteration sustained on 8 axon-tunneled trn2 cores, rel err
3.7e-03 vs the fp32 jax reference (timing via on-device repeat slope; the
axon RPC adds ~70 ms/call that the slope cancels).  Baseline fp32 version of
this same structure measured ~716-838 us; the wins, in order:
  1. bf16 everywhere in the scatter path (PE matmul is 4 cyc/row fp32 vs
     1 cyc/row bf16; DVE gets 2x mode with 2-byte dtypes).  The xw table is
     stored bf16 padded to 128-col rows, so each row is still 256B -- the
     dma_gather minimum descriptor -- and gather/collective traffic is
     UNCHANGED vs fp32 while messages arrive bf16 for free.
  2. Bulk one-hot builds (ohk=16): one DVE tensor_tensor builds 16 one-hot
     groups; the per-group fused tensor_scalar costs ~700ns of fixed
     per-instruction overhead on HW (engine-busy far above the cost model),
     which made the DVE the bottleneck (622us of the 671us fp32 scatter).
     Edge values fold into the messages via one bulk multiply per gather
     batch (emit_scale).
  3. Gather concurrency: maxb=32-group gather batches with msgs_bufs=5 per
     stream -> ~10 dma_gathers in flight over the 4 SWDGE queues.  The
     gather is descriptor-latency bound (~2.3ns/desc = 16 engines x ~37ns
     HBM row cycle; ~107 GB/s effective on random 256B reads) and plateaus
     around 10 in-flight batches.  This is the kernel's floor: ~112k
     descriptors/core = ~250us.
  4. Double-buffered xw_shard/xw_full (rep parity) so a following
     iteration's GEMM + AllGather overlaps the current iteration's scatter
     (removes the WAR serialization between back-to-back invocations).
  5. AllGather split into two chunks (table laid out chunk-major: all cores'
     first half-shards, then second halves); the lo-stream gathers depend
     only on chunk 1 and start while chunk 2 is on the wire.

Structure (per sharding hint):
  - Shard nodes across 8 cores (6250 rows each). Each core computes its
    local xw = x_shard @ W on the PE in bf16 (host pre-transposes x so K
    lands on partitions), stores the 256B-row padded bf16 table, and two
    chunked AllGathers build the full table [50000, 128] in every core's
    Shared DRAM.
  - Edges are partitioned by destination shard on the host, bucketed by
    (dest window of 128 nodes, source half-shard chunk), src-sorted within
    buckets, and padded into 128-edge groups that each target one window.
    Group counts per bucket are padded to the max over cores (~+12%) so all
    8 cores run one SPMD program.  Per group:
      * dma_gather the 128 source rows (256B each) from the bf16 table
      * bulk one-hot [128e, 128d] = is_equal(iota, dst) built 16 groups at
        a time on the DVE; edge vals pre-multiplied into the messages
      * psum[64f, 128d] += msgs[128e, 64f].T @ onehot on the PE (bf16 in,
        fp32 accumulate)
    Window accumulation ends with fused bias+relu on the scalar engine into
    an fp32 staging tile; one DMA stores outT [64, 6250]; host transposes.
  - int16 gather indices address <32768 rows, so the two source chunks (lo =
    all cores' shard rows 0..3124, hi = rows 3125..6249, chunk-major table
    layout) each stay under 25000 rows.
  - Host reassembles: out = concat(outT_c.T).

Do NOT use single_packet=True on the gathers: it hard-crashes the device
(NRT_EXEC_UNIT_UNRECOVERABLE).
"""

import os
import sys

import numpy as np


def _ensure_concourse():
    try:
        import concourse  # noqa: F401
        return
    except ImportError:
        pass
    for p in ("/opt/trn_rl_repo", "/root/.axon_site/_ro/trn_rl_repo"):
        if os.path.isdir(p):
            sys.path.insert(0, p)
            try:
                import concourse  # noqa: F401
                return
            except ImportError:
                sys.path.pop(0)
    raise ImportError("concourse (bass) not found")


_ensure_concourse()

import concourse.bacc as bacc  # noqa: E402
import concourse.mybir as mybir  # noqa: E402
import concourse.tile as tile  # noqa: E402
from concourse import bass_utils  # noqa: E402

F32 = mybir.dt.float32
BF16 = mybir.dt.bfloat16
I16 = mybir.dt.int16
I32 = mybir.dt.int32


def _cdiv(a, b):
    return -(-a // b)


def preprocess(edge_src, edge_dst, edge_vals, *, n_nodes, cores, win,
               src_sort=True):
    """Partition/sort/pad edges into per-core low/high streams.

    Returns a dict with SPMD-uniform structure (G arrays) and per-core data
    arrays laid out exactly as the device consumes them.
    """
    shard = n_nodes // cores
    nwin = _cdiv(shard, win)
    half = n_nodes // 2
    hs = shard // 2

    src = np.asarray(edge_src).astype(np.int64)
    dst = np.asarray(edge_dst).astype(np.int64)
    vals = np.asarray(edge_vals).astype(np.float32)
    e = src.shape[0]

    core = dst // shard
    dl = dst - core * shard
    w = dl // win
    # lo/hi = first/second half of each source core's shard rows; the table
    # is laid out chunk-major (all cores' first halves, then second halves)
    # so the AllGather can be split into two chunks and the lo-stream scatter
    # overlapped with the second chunk.
    sc = src // shard
    sr = src - sc * shard
    h = (sr >= hs).astype(np.int64)
    srcidx = sc * hs + (sr - h * hs)
    key = (core * nwin + w) * 2 + h

    assert shard % 2 == 0
    if src_sort:
        # ascending src within each bucket: the gather's 256B random reads
        # then sweep HBM addresses monotonically per bucket
        order = np.lexsort((srcidx, key))
    else:
        order = np.argsort(key, kind="stable")
    ks = key[order]
    src_s = srcidx[order]
    dloc_s = (dl - w * win)[order].astype(np.float32)
    v_s = vals[order]
    c_s = core[order]
    w_s = w[order]
    h_s = h[order]

    nbuck = cores * nwin * 2
    sizes = np.bincount(key, minlength=nbuck)
    starts = np.concatenate(([0], np.cumsum(sizes)))[:-1]
    rank = np.arange(e, dtype=np.int64) - starts[ks]

    # groups per (window, half): max over cores
    cnt = sizes.reshape(cores, nwin, 2)
    G = _cdiv(cnt, 128).max(axis=0)  # [nwin, 2]
    glo, ghi = G[:, 0], G[:, 1]
    cum_lo = np.concatenate(([0], np.cumsum(glo)))  # group offsets per window
    cum_hi = np.concatenate(([0], np.cumsum(ghi)))
    gtot_lo, gtot_hi = int(cum_lo[-1]), int(cum_hi[-1])
    nlo, nhi = gtot_lo * 128, gtot_hi * 128

    idx_lo = np.zeros((cores, max(nlo, 1)), np.int16)
    dst_lo = np.zeros((cores, max(nlo, 1)), np.float32)
    val_lo = np.zeros((cores, max(nlo, 1)), np.float32)
    idx_hi = np.zeros((cores, max(nhi, 1)), np.int16)
    dst_hi = np.zeros((cores, max(nhi, 1)), np.float32)
    val_hi = np.zeros((cores, max(nhi, 1)), np.float32)

    m = h_s == 0
    pos = cum_lo[w_s[m]] * 128 + rank[m]
    idx_lo[c_s[m], pos] = src_s[m].astype(np.int16)
    dst_lo[c_s[m], pos] = dloc_s[m]
    val_lo[c_s[m], pos] = v_s[m]

    m = h_s == 1
    pos = cum_hi[w_s[m]] * 128 + rank[m]
    idx_hi[c_s[m], pos] = src_s[m].astype(np.int16)
    dst_hi[c_s[m], pos] = dloc_s[m]
    val_hi[c_s[m], pos] = v_s[m]

    def idx_layout(a, n):
        # logical position i -> [i % 16, i // 16], replicated to 128 partitions
        if n == 0:
            return None
        blk = a[:n].reshape(-1, 16).T  # [16, n/16]
        return np.ascontiguousarray(np.tile(blk, (8, 1)))  # [128, n/16]

    def grp_layout(a, n):
        # position g*128+p -> [p, g]
        if n == 0:
            return None
        return np.ascontiguousarray(a[:n].reshape(-1, 128).T)  # [128, G]

    return dict(
        shard=shard,
        nwin=nwin,
        half=half,
        glo=glo,
        ghi=ghi,
        cum_lo=cum_lo,
        cum_hi=cum_hi,
        gtot_lo=gtot_lo,
        gtot_hi=gtot_hi,
        nlo=nlo,
        nhi=nhi,
        idx_lo=[idx_layout(idx_lo[c], nlo) for c in range(cores)],
        dst_lo=[grp_layout(dst_lo[c], nlo) for c in range(cores)],
        val_lo=[grp_layout(val_lo[c], nlo) for c in range(cores)],
        idx_hi=[idx_layout(idx_hi[c], nhi) for c in range(cores)],
        dst_hi=[grp_layout(dst_hi[c], nhi) for c in range(cores)],
        val_hi=[grp_layout(val_hi[c], nhi) for c in range(cores)],
    )


def build_program(meta, *, n_nodes, din, dout, cores, win, maxb=64,
                  msgs_bufs=2, gp_build=0, sc_ps_bufs=4, debug_mode=None,
                  ohk=16, oh_bufs=3, act_build=0, single_packet=False,
                  debug_skip_gemm=False, debug_skip_scatter=False, repeat=1):
    """Build the SPMD Bass program. Returns (nc, input_names)."""
    shard = meta["shard"]
    nwin = meta["nwin"]
    half = meta["half"]
    glo, ghi = meta["glo"], meta["ghi"]
    cum_lo, cum_hi = meta["cum_lo"], meta["cum_hi"]
    gtot_lo, gtot_hi = meta["gtot_lo"], meta["gtot_hi"]
    nlo, nhi = meta["nlo"], meta["nhi"]
    kch = _cdiv(din, 128)
    nr = _cdiv(shard, 128)

    # padded bf16 table row: dout reals + pad to 256B so dma_gather's
    # 256B-stride restriction is met; gather traffic is unchanged vs fp32
    # but messages arrive bf16 (PE 4x, DVE 2x vs fp32).
    drow = 128
    assert dout <= drow

    nc = bacc.Bacc("TRN2", target_bir_lowering=False, debug=False,
                   num_devices=cores, num_swdge_queues=4)

    t_xT = nc.dram_tensor("xT", [din, shard], BF16, kind="ExternalInput")
    t_w = nc.dram_tensor("w", [din, dout], BF16, kind="ExternalInput")
    t_bias = nc.dram_tensor("bias", [dout, 1], F32, kind="ExternalInput")
    t_idx = {}
    t_dst = {}
    t_val = {}
    if nlo:
        t_idx["lo"] = nc.dram_tensor("idx_lo", [128, nlo // 16], I16, kind="ExternalInput")
        t_dst["lo"] = nc.dram_tensor("dst_lo", [128, gtot_lo], F32, kind="ExternalInput")
        t_val["lo"] = nc.dram_tensor("val_lo", [128, gtot_lo], F32, kind="ExternalInput")
    if nhi:
        t_idx["hi"] = nc.dram_tensor("idx_hi", [128, nhi // 16], I16, kind="ExternalInput")
        t_dst["hi"] = nc.dram_tensor("dst_hi", [128, gtot_hi], F32, kind="ExternalInput")
        t_val["hi"] = nc.dram_tensor("val_hi", [128, gtot_hi], F32, kind="ExternalInput")
    t_out = nc.dram_tensor("outT", [dout, shard], F32, kind="ExternalOutput")

    if debug_skip_gemm == "shared":
        # probe: gather from a Shared-scratchpad tensor filled by DMA copy
        t_xw_in = nc.dram_tensor("xw_full", [n_nodes, drow], BF16,
                                 kind="ExternalInput")
        t_xw_full = nc.dram_tensor("xw_full_sh", [n_nodes, drow], BF16,
                                   addr_space="Shared")
    elif debug_skip_gemm:
        t_xw_full = nc.dram_tensor("xw_full", [n_nodes, drow], BF16,
                                   kind="ExternalInput")
    else:
        # double-buffered by rep parity so iteration k+1's GEMM+AllGather
        # overlaps iteration k's scatter (no WAR serialization between
        # back-to-back invocations)
        nbuf = min(repeat, 2)
        t_xw_shard = [nc.dram_tensor(f"xw_shard{i}", [shard, drow], BF16)
                      for i in range(nbuf)]
        t_xw_full = [nc.dram_tensor(f"xw_full{i}", [n_nodes, drow], BF16,
                                    addr_space="Shared" if cores > 4 else "Local")
                     for i in range(nbuf)]

    with tile.TileContext(nc) as tc:
        with (
            tc.tile_pool(name="const", bufs=1) as constp,
            tc.tile_pool(name="xt", bufs=1) as xtp,
            tc.tile_pool(name="stage", bufs=1) as stagep,
            tc.tile_pool(name="meta", bufs=1) as metap,
            tc.tile_pool(name="msgs_lo", bufs=msgs_bufs) as mlp,
            tc.tile_pool(name="msgs_hi", bufs=msgs_bufs) as mhp,
            tc.tile_pool(name="oh", bufs=oh_bufs) as ohp,
            tc.tile_pool(name="gemm_ps", bufs=2, space="PSUM") as gpsp,
            tc.tile_pool(name="sc_ps", bufs=sc_ps_bufs, space="PSUM") as spsp,
        ):
            # ---- constants ----
            # iota repeated max(ohk,1) times: iota_rep[p, k*win + j] = j
            iw = max(ohk, 1)
            iota_i = constp.tile([128, iw * win], I32)
            nc.gpsimd.iota(iota_i[:], pattern=[[0, iw], [1, win]], base=0,
                           channel_multiplier=0)
            iota_f = constp.tile([128, iw * win], BF16)
            nc.vector.tensor_copy(iota_f[:], iota_i[:])
            bias_sb = constp.tile([dout, 1], F32)
            nc.sync.dma_start(bias_sb[:], t_bias[:])
            w_sb = constp.tile([128, kch * dout], BF16)
            nc.sync.dma_start(
                w_sb[:].rearrange("p (k e) -> p k e", e=dout),
                t_w[:].rearrange("(k p) e -> p k e", p=128),
            )

            # ---- metadata loads ----
            sb_idx = {}
            sb_dst = {}
            sb_val = {}
            sb_dst16 = {}
            sb_val16 = {}
            for s in t_idx:
                sb_idx[s] = metap.tile(list(t_idx[s].shape), I16, tag=f"idx_{s}", name=f"sb_idx_{s}")
                nc.sync.dma_start(sb_idx[s][:], t_idx[s][:])
                sb_dst[s] = metap.tile(list(t_dst[s].shape), F32, tag=f"dst_{s}", name=f"sb_dst_{s}")
                nc.sync.dma_start(sb_dst[s][:], t_dst[s][:])
                sb_val[s] = metap.tile(list(t_val[s].shape), F32, tag=f"val_{s}", name=f"sb_val_{s}")
                nc.sync.dma_start(sb_val[s][:], t_val[s][:])
                if ohk:
                    sb_dst16[s] = metap.tile(list(t_dst[s].shape), BF16,
                                             tag=f"dst16_{s}", name=f"sb_dst16_{s}")
                    nc.vector.tensor_copy(sb_dst16[s][:], sb_dst[s][:])
                    sb_val16[s] = metap.tile(list(t_val[s].shape), BF16,
                                             tag=f"val16_{s}", name=f"sb_val16_{s}")
                    nc.vector.tensor_copy(sb_val16[s][:], sb_val[s][:])
            sb_negd = {}
            sb_negv = {}
            if act_build:
                for s in t_idx:
                    sb_negd[s] = metap.tile(list(t_dst[s].shape), F32,
                                            tag=f"negd_{s}", name=f"sb_negd_{s}")
                    nc.vector.tensor_scalar(
                        sb_negd[s][:], sb_dst[s][:], -1.0, None,
                        mybir.AluOpType.mult)
                    sb_negv[s] = metap.tile(list(t_val[s].shape), F32,
                                            tag=f"negv_{s}", name=f"sb_negv_{s}")
                    nc.vector.tensor_scalar(
                        sb_negv[s][:], sb_val[s][:], -1.0, None,
                        mybir.AluOpType.mult)

            # ---- per-iteration body (repeat>1 used only for timing) ----
            def emit_body(rep):
                # ---- local GEMM: xw_shard = x_shard @ W ----
                if debug_skip_gemm == "shared":
                    xt_sb = None
                    r_xw_full = t_xw_full
                    nc.sync.dma_start(t_xw_full[:], t_xw_in[:])
                elif debug_skip_gemm:
                    xt_sb = None
                    r_xw_full = t_xw_full
                else:
                    r_xw_shard = t_xw_shard[rep % len(t_xw_shard)]
                    r_xw_full = t_xw_full[rep % len(t_xw_full)]
                    xt_sb = []
                    for k in range(kch):
                        kp = min(128, din - k * 128)
                        xt = xtp.tile([kp, shard], BF16, tag=f"xt{k}")
                        nc.sync.dma_start(xt[:], t_xT[k * 128:k * 128 + kp, :])
                        xt_sb.append(xt)
                    xw_stage = stagep.tile([128, nr * drow], BF16, tag="xw_stage")
                    # pad cols are never read by the matmuls, but zero them so
                    # the gathered bytes are defined for the simulator
                    nc.vector.memset(
                        xw_stage[:].rearrange("p (r e) -> p r e", e=drow)[:, :, dout:],
                        0.0)
                    for r in range(nr):
                        rw = min(128, shard - r * 128)
                        ps = gpsp.tile([rw, dout], F32, tag="gemm_ps")
                        for k in range(kch):
                            nc.tensor.matmul(
                                ps[:],
                                xt_sb[k][:, r * 128:r * 128 + rw],
                                w_sb[:xt_sb[k].shape[0], k * dout:(k + 1) * dout],
                                start=(k == 0),
                                stop=(k == kch - 1),
                            )
                        nc.scalar.activation(
                        xw_stage[:rw, r * drow:r * drow + dout], ps[:],
                        mybir.ActivationFunctionType.Copy)
                    # store xw_shard (row-major, 256B padded rows) then AllGather
                    nfull = shard // 128
                    nc.sync.dma_start(
                        r_xw_shard[: nfull * 128, :].rearrange("(r p) e -> p r e", p=128),
                        xw_stage[:, : nfull * drow].rearrange("p (r e) -> p r e", e=drow),
                    )
                    if shard > nfull * 128:
                        rw = shard - nfull * 128
                        nc.sync.dma_start(
                            r_xw_shard[nfull * 128:, :],
                            xw_stage[:rw, nfull * drow:(nfull + 1) * drow],
                        )
                    # chunked AllGather: the lo-stream scatter only needs the
                    # first chunk, so its gathers overlap the second chunk
                    hs = shard // 2
                    nc.gpsimd.collective_compute(
                        "AllGather",
                        mybir.AluOpType.bypass,
                        replica_groups=[list(range(cores))],
                        ins=[r_xw_shard[0:hs, :]],
                        outs=[r_xw_full[0:cores * hs, :]],
                    )
                    nc.gpsimd.collective_compute(
                        "AllGather",
                        mybir.AluOpType.bypass,
                        replica_groups=[list(range(cores))],
                        ins=[r_xw_shard[hs:shard, :]],
                        outs=[r_xw_full[cores * hs:n_nodes, :]],
                    )

                # ---- scatter phase ----
                in_ap = {}
                if nlo:
                    in_ap["lo"] = r_xw_full[0:half, :]
                if nhi:
                    in_ap["hi"] = r_xw_full[half:n_nodes, :]
                gtot = {"lo": gtot_lo, "hi": gtot_hi}
                nbatch = {s: _cdiv(gtot[s], maxb) for s in in_ap}
                pool = {"lo": mlp, "hi": mhp}
                msgs_buf = {s: [None] * nbatch[s] for s in in_ap}
                qctr = [0]
                noh = {s: _cdiv(gtot[s], max(ohk, 1)) for s in in_ap}
                oh_buf = {s: [None] * noh[s] for s in in_ap}
                scaled = {s: [False] * nbatch[s] for s in in_ap}

                def emit_oh(s, c):
                    g0 = c * ohk
                    gn = min(ohk, gtot[s] - g0)
                    buf = ohp.tile([128, gn * win], BF16, tag="oh",
                                   name=f"oh_{s}_{c}_r{rep}")
                    nc.vector.tensor_tensor(
                        buf[:].rearrange("p (k j) -> p k j", j=win),
                        iota_f[:, :gn * win].rearrange("p (k j) -> p k j", j=win),
                        sb_dst16[s][:, g0:g0 + gn].broadcast_to([128, gn, win]),
                        op=mybir.AluOpType.is_equal,
                    )
                    oh_buf[s][c] = buf

                def emit_gather(s, b):
                    g0 = b * maxb
                    gn = min(maxb, gtot[s] - g0)
                    n_idx = gn * 128
                    buf = pool[s].tile([128, gn * drow], BF16, tag=f"msgs_{s}", name=f"msgs_{s}_{b}_r{rep}")
                    nc.gpsimd.dma_gather(
                        buf[:].rearrange("p (c e) -> p c e", e=drow),
                        in_ap[s],
                        sb_idx[s][:, g0 * 8:(g0 + gn) * 8],
                        n_idx,
                        n_idx,
                        drow,
                        single_packet=single_packet,
                        queue_num=qctr[0] % 4,
                    )
                    qctr[0] += 1
                    msgs_buf[s][b] = buf

                def emit_scale(s, b):
                    # fold edge values into the messages: one bulk multiply.
                    # Deferred to first consumption so the DVE doesn't
                    # head-of-line block on the gather DMA.
                    g0 = b * maxb
                    gn = min(maxb, gtot[s] - g0)
                    buf = msgs_buf[s][b]
                    nc.vector.tensor_tensor(
                        buf[:].rearrange("p (c e) -> p c e", e=drow)[:, :, :dout],
                        buf[:].rearrange("p (c e) -> p c e", e=drow)[:, :, :dout],
                        sb_val16[s][:, g0:g0 + gn].broadcast_to([128, gn, dout]),
                        op=mybir.AluOpType.mult,
                    )
                    scaled[s][b] = True

                out_stage = stagep.tile([dout, shard], F32, tag="out_stage")
                cum = {"lo": cum_lo, "hi": cum_hi}
                if debug_skip_scatter == "gather_only":
                    # gathers only; consume each batch with one cheap DVE add
                    acc = stagep.tile([128, dout], BF16, tag="dbg_acc")
                    nc.vector.memset(acc[:], 0.0)
                    for s in in_ap:
                        for b in range(nbatch[s]):
                            emit_gather(s, b)
                            nc.vector.tensor_tensor(
                                acc[:], acc[:], msgs_buf[s][b][:, :dout],
                                op=mybir.AluOpType.add)
                    nc.vector.tensor_copy(out_stage[:, :dout], acc[:dout, :dout])
                    nc.sync.dma_start(t_out[:], out_stage[:])
                    return
                if debug_skip_scatter:
                    # debug: consume a slice of xw_full so the collective is
                    # on the critical path; output is meaningless
                    nrows = (shard // 128) * 128
                    dbg = stagep.tile([128, nrows], BF16, tag="dbg_stage")
                    nc.sync.dma_start(
                        dbg[:].rearrange("p (r e) -> p r e", e=drow),
                        r_xw_full[0:nrows, :].rearrange("(r p) e -> p r e", p=128))
                    nc.vector.tensor_copy(
                        out_stage[:, :nrows], dbg[:dout, :nrows])
                    nc.sync.dma_start(t_out[:], out_stage[:])
                    nwin_eff = 0
                else:
                    nwin_eff = nwin
                def emit_chain(s, g0, g1, ps, gi0=0, ngrp=None):
                    if ngrp is None:
                        ngrp = g1 - g0
                    for gi, g in enumerate(range(g0, g1), start=gi0):
                        b, j = g // maxb, g % maxb
                        if msgs_buf[s][b] is None:
                            emit_gather(s, b)
                        if debug_mode == "const_oh":
                            oh_ap = iota_f[:, :win]
                        elif ohk == 0:
                            oh = ohp.tile([128, win], BF16, tag="oh")
                            if act_build and gi % act_build == act_build - 1:
                                # scalar-engine build (exact for int iota):
                                # oh = val * relu(1 - |iota - dst|)
                                ab = ohp.tile([128, win], BF16, tag="abst")
                                nc.scalar.activation(
                                    ab[:], iota_f[:, :win],
                                    mybir.ActivationFunctionType.Abs,
                                    bias=sb_negd[s][:, g:g + 1])
                                nc.scalar.activation(
                                    oh[:], ab[:],
                                    mybir.ActivationFunctionType.Relu,
                                    bias=sb_val[s][:, g:g + 1],
                                    scale=sb_negv[s][:, g:g + 1])
                            else:
                                # per-group fused build on DVE
                                nc.vector.tensor_scalar(
                                    oh[:],
                                    iota_f[:, :win],
                                    sb_dst[s][:, g:g + 1],
                                    sb_val[s][:, g:g + 1],
                                    mybir.AluOpType.is_equal,
                                    mybir.AluOpType.mult,
                                )
                            oh_ap = oh[:]
                        else:
                            if not scaled[s][b]:
                                emit_scale(s, b)
                            oc, ojj = g // ohk, g % ohk
                            if oh_buf[s][oc] is None:
                                emit_oh(s, oc)
                            oh_ap = oh_buf[s][oc][:, ojj * win:(ojj + 1) * win]
                        if debug_mode == "no_mm":
                            if gi == 0:
                                nc.tensor.matmul(
                                    ps[:], msgs_buf[s][b][:, j * drow:j * drow + dout],
                                    oh_ap, start=True, stop=True)
                        else:
                            nc.tensor.matmul(
                                ps[:],
                                msgs_buf[s][b][:, j * drow:j * drow + dout],
                                oh_ap,
                                start=(gi == 0),
                                stop=(gi == ngrp - 1),
                            )

                def wspan(s, wi):
                    if s not in in_ap:
                        return 0, 0
                    return int(cum[s][wi]), int(cum[s][wi + 1])

                for wi in range(nwin_eff):
                    ww = min(win, shard - wi * win)
                    sl = slice(wi * win, wi * win + ww)
                    spans = [(s, *wspan(s, wi)) for s in in_ap]
                    ngrp = sum(g1 - g0 for _, g0, g1 in spans)
                    if ngrp == 0:
                        # no edges anywhere for this window: bias + relu of zero
                        zps = spsp.tile([dout, win], F32, tag="sc_ps")
                        nc.vector.memset(zps[:], 0.0)
                        nc.scalar.activation(
                            out_stage[:, sl], zps[:, :ww],
                            mybir.ActivationFunctionType.Relu, bias=bias_sb[:],
                        )
                        continue
                    ps = spsp.tile([dout, win], F32, tag="sc_ps")
                    gi = 0
                    for s, g0, g1 in spans:
                        if g1 > g0:
                            emit_chain(s, g0, g1, ps, gi, ngrp)
                            gi += g1 - g0
                    nc.scalar.activation(
                        out_stage[:, sl], ps[:, :ww],
                        mybir.ActivationFunctionType.Relu, bias=bias_sb[:],
                    )
                nc.sync.dma_start(t_out[:], out_stage[:])

            for _rep in range(repeat):
                emit_body(_rep)

    nc.compile()
    return nc


def run(inputs, *, n_nodes, n_edges, din, dout, cores, win=128, maxb=64,
        msgs_bufs=2, gp_build=0, sc_ps_bufs=4, debug_mode=None,
        ohk=16, oh_bufs=3, act_build=0, single_packet=False, src_sort=True,
        trace=False, debug_skip_gemm=False, debug_skip_scatter=False,
        repeat=1):
    import ml_dtypes

    x = np.ascontiguousarray(np.asarray(inputs["x"], dtype=np.float32))
    weight = np.ascontiguousarray(np.asarray(inputs["weight"], dtype=np.float32))
    bias = np.ascontiguousarray(
        np.asarray(inputs["bias"], dtype=np.float32).reshape(dout, 1))
    meta = preprocess(
        inputs["edge_src"], inputs["edge_dst"], inputs["edge_vals"],
        n_nodes=n_nodes, cores=cores, win=win, src_sort=src_sort)
    shard = meta["shard"]

    nc = build_program(meta, n_nodes=n_nodes, din=din, dout=dout, cores=cores,
                       win=win, maxb=maxb, msgs_bufs=msgs_bufs,
                       gp_build=gp_build, sc_ps_bufs=sc_ps_bufs,
                       debug_mode=debug_mode, ohk=ohk, oh_bufs=oh_bufs,
                       act_build=act_build, single_packet=single_packet,
                       debug_skip_gemm=debug_skip_gemm,
                       debug_skip_scatter=debug_skip_scatter, repeat=repeat)

    xT = np.ascontiguousarray(x.T.astype(ml_dtypes.bfloat16))
    w_bf = np.ascontiguousarray(weight.astype(ml_dtypes.bfloat16))
    in_maps = []
    for c in range(cores):
        m = {
            "xT": np.ascontiguousarray(xT[:, c * shard:(c + 1) * shard]),
            "w": w_bf,
            "bias": bias,
        }
        if meta["nlo"]:
            m["idx_lo"] = meta["idx_lo"][c]
            m["dst_lo"] = meta["dst_lo"][c]
            m["val_lo"] = meta["val_lo"][c]
        if meta["nhi"]:
            m["idx_hi"] = meta["idx_hi"][c]
            m["dst_hi"] = meta["dst_hi"][c]
            m["val_hi"] = meta["val_hi"][c]
        if debug_skip_gemm:
            xw = (x @ weight).astype(ml_dtypes.bfloat16)
            xw_pad = np.zeros((n_nodes, 128), ml_dtypes.bfloat16)
            xw_pad[:, :dout] = xw
            m["xw_full"] = xw_pad
        in_maps.append(m)

    res = bass_utils.run_bass_kernel_spmd(
        nc, in_maps, core_ids=list(range(cores)), trace=trace)
    out = np.concatenate(
        [res.results[c]["outT"].T for c in range(cores)], axis=0)
    run.last_nc = nc
    run.last_in_maps = in_maps
    return out, res


def kernel(**inputs):
    out, _ = run(
        inputs,
        n_nodes=50000, n_edges=800000, din=256, dout=64, cores=8,
        maxb=32, msgs_bufs=5, ohk=16, oh_bufs=3,
    )
    return np.ascontiguousarray(out, dtype=np.float32)

